# revision 12
# baseline (speedup 1.0000x reference)
"""Causal attention block (QKV proj + RoPE + causal SDPA + out proj) on 8
Trainium2 NeuronCores.

Sharding: core c = 4*b + g handles batch b (of 2) and head group g (of 4,
4 heads each).  Each core computes q/k/v for its 4 heads from x[b] and the
matching Wqkv column slices, runs causal SDPA, and contracts its 512
output-feature rows of Wproj, producing a partial projB [2048(tok),
2048(oc)].  The host sums the 4 partials per batch.

v2 design notes (vs the fp32r baseline):
  * All matmul operands are bf16 (PSUM accumulation stays fp32).  bf16
    stationaries enable Fast Weight Load (2 cols/cycle) -- fp32r LDWEIGHTS
    measured ~190ns/tile and made QKV LDW-port-bound (~224ns/MM observed vs
    160ns stream-ideal).  End-to-end bf16 error measured 3.9e-3 <= 2e-2.
  * Softmax row sums are fused into the attn@v matmuls: e-tiles are the
    STATIONARY operand ([128 keys, 128 queries] chunks) and the moving
    operand is v with an appended all-ones column [128 keys, 129].  The
    PSUM result is [queries, hd | rowsum], so the denominator lands as a
    per-partition scalar: reciprocal on [128,1] + tensor_scalar broadcast.
    This removes the separate all-ones rowsum matmul (1/3 of attention
    matmul rows in the baseline).
  * The [q, hd] attention output is transposed back to [hd, q] for the
    projection with SBUF->SBUF dma_start_transpose (xbar), costing no
    engine time.
  * Projection emits projB[tok, oc] (stationary = outT chunk, moving = Wproj
    rows), so neither device nor host transposes the output; output is bf16
    (halves the output DMA).
  * exp runs once per (panel, jb) over both heads' scores ([128, 2, 512-n0]
    strided PSUM read) halving ACT instruction overheads.
  * RoPE reads q/k PSUM directly (swap-halves via partition-offset ACT
    copies + in-place DVE muls); q/k/v PSUM banks free early so the next
    panel's matmuls are never blocked on the RoPE chain.
"""

import sys

if "/opt/trn_rl_repo" not in sys.path:
    sys.path.insert(0, "/opt/trn_rl_repo")

from contextlib import ExitStack

import ml_dtypes
import numpy as np

import concourse.bass as bass  # noqa: F401
import concourse.tile as tile
from concourse import bacc, bass_utils, mybir

F32 = mybir.dt.float32
BF16 = mybir.dt.bfloat16
EXP = mybir.ActivationFunctionType.Exp

B, N, C = 2, 2048, 2048
H = 16  # total heads
HD = C // H  # 128
G = 4  # head groups (cores per batch)
HPG = H // G  # 4 heads per group
P = 128
PANEL = 512
NP = N // PANEL  # 4 token panels
KB = C // P  # 16 contraction blocks
NJB = N // P  # 16 key blocks
SCALE = float(HD) ** -0.5
ROPE_BASE = 10000.0

_NC_CACHE = {}
DEBUG = False


def _bc2(ap, n=2):
    """Broadcast a [128, F] AP across an inserted middle dim -> [128, n, F]."""
    p, f = ap.shape
    return ap.rearrange("p (o n) -> p o n", o=1).broadcast_to([p, n, f])


def _emit(ctx, tc, t):
    nc = tc.nc
    vec = nc.vector
    sca = nc.scalar
    mm = nc.tensor.matmul

    const = ctx.enter_context(tc.tile_pool(name="const", bufs=1))
    wpool = ctx.enter_context(tc.tile_pool(name="w", bufs=2))
    xpool = ctx.enter_context(tc.tile_pool(name="x", bufs=2))
    qkpool = ctx.enter_context(tc.tile_pool(name="qk", bufs=2))
    vpool = ctx.enter_context(tc.tile_pool(name="v", bufs=2))
    rpool = ctx.enter_context(tc.tile_pool(name="rope", bufs=2))
    epool = ctx.enter_context(tc.tile_pool(name="e", bufs=4))
    opool = ctx.enter_context(tc.tile_pool(name="o", bufs=1))
    pnpool = ctx.enter_context(tc.tile_pool(name="pn", bufs=2))
    rspool = ctx.enter_context(tc.tile_pool(name="rs", bufs=4))
    poutp = ctx.enter_context(tc.tile_pool(name="pout", bufs=4))
    ps = ctx.enter_context(tc.tile_pool(name="ps", bufs=1, space="PSUM"))

    cosT = const.tile([P, N], F32)
    sinT = const.tile([P, N], F32)
    tri = const.tile([P, P], BF16)
    zeros = const.tile([P, 264], BF16)
    # consts go on the gpsimd DMA queue so they don't delay the first
    # weight/x loads on the sync queue
    nc.gpsimd.dma_start(cosT, t["cosT"])
    nc.gpsimd.dma_start(sinT, t["sinT"])
    nc.gpsimd.dma_start(tri, t["tri"])
    vec.memset(zeros, 0.0)

    # wp4 loaded later (during sweep 0) to keep the startup DMA short
    wp4 = const.tile([P, HPG, N], BF16, name="wp4")

    outT = [
        opool.tile([P, N], BF16, tag=f"outT{h}", name=f"outT{h}")
        for h in range(HPG)
    ]

    xT3 = t["xT"].rearrange("(kb q) n -> q kb n", q=P)

    def rope(psrc, dst, sl):
        """dst[:, :, sl] = psum*cos + swap64(psum)*sin'  (sin' pre-negated
        on partitions 0-63).  One fast ACT copy frees the PSUM bank; the
        swap copies run on the otherwise-idle GpSimd engine."""
        raw = rpool.tile([P, 2, PANEL], F32, tag="raw", name="raw")
        sca.copy(raw, psrc)
        rw = rpool.tile([P, 2, PANEL], F32, tag="rw", name="rw")
        nc.gpsimd.tensor_copy(rw[0:64], raw[64:128])
        nc.gpsimd.tensor_copy(rw[64:128], raw[0:64])
        vec.tensor_mul(rw, rw, _bc2(sinT[:, sl]))
        vec.tensor_mul(raw, raw, _bc2(cosT[:, sl]))
        vec.tensor_add(dst[:, :, sl], raw, rw)

    def load_w(sweep):
        """Per-sweep 256-col slices of wq/wk/wv -> [128, KB, 256] bf16."""
        w_sb = {}
        for wname in ("wq", "wk", "wv"):
            w3 = t[wname].rearrange("(kb p) f -> p kb f", p=P)
            # split into halves so the first matmuls can start early
            w_t = wpool.tile([P, KB, 256], BF16, tag=wname, name=wname)
            for hf in range(2):
                nc.sync.dma_start(
                    w_t[:, 8 * hf : 8 * hf + 8, :],
                    w3[:, 8 * hf : 8 * hf + 8, 256 * sweep : 256 * sweep + 256],
                )
            w_sb[wname] = w_t
        return w_sb

    for sweep in range(2):
        w_sb = load_w(sweep) if sweep == 0 else w_next  # noqa: F821
        # per-sweep state
        qq = qkpool.tile([P, 2, N], BF16, tag="qq", name="qq")
        kk = qkpool.tile([P, 2, N], BF16, tag="kk", name="kk")
        v_sb = vpool.tile([P, NJB, 2, 132], BF16, tag="v", name="v_sb")
        vec.memset(v_sb[:, :, :, 128:129], 1.0)

        # ---- phase A: QKV + RoPE for this sweep's 2 heads ----
        for p in range(NP):
            sl = slice(PANEL * p, PANEL * (p + 1))
            pq01 = ps.tile([P, 2, PANEL], F32, tag="SC0", name="pq01")
            pk01 = ps.tile([P, 2, PANEL], F32, tag="SC1", name="pk01")
            pv = [
                ps.tile([P, 256], F32, tag=f"PO{tb}", name=f"pv{tb}")
                for tb in range(4)
            ]
            xt = [None, None]
            for hb in range(2):
                xt[hb] = xpool.tile([P, KB // 2, PANEL], BF16, tag="x", name="xt")
                nc.sync.dma_start(xt[hb], xT3[:, 8 * hb : 8 * hb + 8, sl])
            # all-q, then all-k, then all-v: the q/k PSUM banks drain while
            # the later groups stream, so the next panel never stalls on RoPE
            for wn, pdst in (("wq", pq01), ("wk", pk01)):
                for kb in range(KB):
                    st, sp = kb == 0, kb == KB - 1
                    x_k = xt[kb // 8][:, kb % 8]
                    mm(pdst[:, 0], w_sb[wn][:, kb, 0:128], x_k, start=st, stop=sp)
                    mm(pdst[:, 1], w_sb[wn][:, kb, 128:256], x_k, start=st, stop=sp)
                if wn == "wq":
                    rope(pq01, qq, sl)
            rope(pk01, kk, sl)
            for kb in range(KB):
                st, sp = kb == 0, kb == KB - 1
                x_k = xt[kb // 8][:, kb % 8]
                for tb in range(4):
                    mm(
                        pv[tb],
                        x_k[:, 128 * tb : 128 * (tb + 1)],
                        w_sb["wv"][:, kb],
                        start=st,
                        stop=sp,
                    )
            for tb in range(4):
                sca.copy(
                    v_sb[:, 4 * p + tb, :, 0:128],
                    pv[tb].rearrange("p (h f) -> p h f", h=2),
                )

        # prefetch next sweep's weights / wp4 during attention
        if sweep == 0:
            w_next = load_w(1)
            nc.sync.dma_start(wp4, t["wp4"])

        # ---- phase B: causal SDPA (+ proj on sweep 1) ----
        for p in range(NP):
            njb = 4 * p + 4
            po = {
                (hh, pair): ps.tile(
                    [P, 2, 132], F32, tag=f"PO{2 * hh + pair}", name="po"
                )
                for hh in range(2)
                for pair in range(2)
            }
            # Two accumulation groups share each po bank, but a start=True
            # matmul clears has_written for the WHOLE bank.  So pre-zero the
            # bank with one dummy matmul (sets has_written everywhere) and
            # accumulate with start=False.
            for key in po:
                mm(
                    po[key].rearrange("p a b -> p (a b)"),
                    tri,
                    zeros,
                    start=True,
                    stop=False,
                    skip_group_check=True,
                )
            e_tiles = []

            def emit_av(jj):
                e1, td = e_tiles[jj]
                for hh in range(2):
                    for qc in range(max(0, td), 4):
                        mm(
                            po[hh, qc // 2][:, qc % 2, 0:129],
                            e1[:, hh, 128 * qc : 128 * (qc + 1)],
                            v_sb[:, jj, hh, 0:129],
                            start=False,
                            stop=(jj == 4 * p + qc),
                            skip_group_check=True,
                        )

            po_n = {
                hh: pnpool.tile([P, PANEL], BF16, tag=f"pn{hh}", name="po_n")
                for hh in range(2)
            }

            def normalize(qc):
                # qc's accumulation finished at jj = 4p+qc: divide by the
                # fused rowsum (col 128) and transpose back, immediately, so
                # the po bank frees early and the transpose DMA overlaps the
                # rest of the panel.
                for hh in range(2):
                    src = po[hh, qc // 2][:, qc % 2]
                    rs_rec = rspool.tile([P, 1], F32, tag="rs", name="rs_rec")
                    vec.reciprocal(rs_rec, src[:, 128:129])
                    vec.tensor_scalar_mul(
                        po_n[hh][:, 128 * qc : 128 * (qc + 1)], src[:, 0:128], rs_rec
                    )
                    dma_eng = nc.sync if hh == 0 else nc.scalar
                    dma_eng.dma_start_transpose(
                        outT[2 * sweep + hh][
                            :, PANEL * p + 128 * qc : PANEL * p + 128 * (qc + 1)
                        ],
                        po_n[hh][:, 128 * qc : 128 * (qc + 1)],
                    )

            def after_av(jj):
                qc = jj - 4 * p
                if 0 <= qc <= 3:
                    normalize(qc)

            for jj in range(njb):
                td = jj - 4 * p  # diagonal sub-block index if >= 0
                n0 = 128 * td if td > 0 else 0
                sc = ps.tile([P, 2, PANEL], F32, tag=f"SC{jj % 2}", name="sc")
                for hh in range(2):
                    mm(
                        sc[:, hh, n0:],
                        kk[:, hh, 128 * jj : 128 * (jj + 1)],
                        qq[:, hh, PANEL * p + n0 : PANEL * (p + 1)],
                    )
                e1 = epool.tile([P, 2, PANEL], BF16, tag="e1", name="e1")
                sca.activation(e1[:, :, n0:], sc[:, :, n0:], EXP, scale=SCALE)
                if td >= 0:
                    dsl = slice(128 * td, 128 * (td + 1))
                    vec.tensor_mul(e1[:, :, dsl], e1[:, :, dsl], _bc2(tri))
                e_tiles.append((e1, td))
                if jj >= 2:
                    emit_av(jj - 2)
                    after_av(jj - 2)
            emit_av(njb - 2)
            after_av(njb - 2)
            emit_av(njb - 1)
            after_av(njb - 1)

            if sweep == 1:
                # proj for this panel: outT[0..3][:, psl] are final now
                for tc in range(4):
                    tsl = slice(PANEL * p + 128 * tc, PANEL * p + 128 * (tc + 1))
                    for occ in range(4):
                        pj = ps.tile(
                            [P, PANEL], F32, tag=f"SC{occ % 2}", name="pj"
                        )
                        osl = slice(PANEL * occ, PANEL * (occ + 1))
                        for h in range(HPG):
                            mm(
                                pj,
                                outT[h][:, tsl],
                                wp4[:, h, osl],
                                start=(h == 0),
                                stop=(h == HPG - 1),
                            )
                        o_t = poutp.tile([P, PANEL], BF16, tag="pout", name="o_t")
                        if occ % 2 == 0:
                            sca.copy(o_t, pj)
                        else:
                            vec.tensor_copy(o_t, pj)
                        nc.sync.dma_start(t["projB"][tsl, osl], o_t)

    if DEBUG:
        for h in range(HPG):
            nc.sync.dma_start(t[f"dbg_o{h}"], outT[h])


def build_nc():
    key = (DEBUG,)
    if key in _NC_CACHE:
        return _NC_CACHE[key]
    nc = bacc.Bacc("TRN2", target_bir_lowering=False, debug=False)
    t = {}
    t["xT"] = nc.dram_tensor("xT", [C, N], BF16, kind="ExternalInput").ap()
    t["wq"] = nc.dram_tensor("wq", [C, 512], BF16, kind="ExternalInput").ap()
    t["wk"] = nc.dram_tensor("wk", [C, 512], BF16, kind="ExternalInput").ap()
    t["wv"] = nc.dram_tensor("wv", [C, 512], BF16, kind="ExternalInput").ap()
    t["wp4"] = nc.dram_tensor("wp4", [P, HPG, N], BF16, kind="ExternalInput").ap()
    t["cosT"] = nc.dram_tensor("cosT", [P, N], F32, kind="ExternalInput").ap()
    t["sinT"] = nc.dram_tensor("sinT", [P, N], F32, kind="ExternalInput").ap()
    t["tri"] = nc.dram_tensor("tri", [P, P], BF16, kind="ExternalInput").ap()
    t["projB"] = nc.dram_tensor("projB", [N, N], BF16, kind="ExternalOutput").ap()
    if DEBUG:
        for h in range(HPG):
            t[f"dbg_o{h}"] = nc.dram_tensor(
                f"dbg_o{h}", [P, N], BF16, kind="ExternalOutput"
            ).ap()
    with tile.TileContext(nc) as tc, ExitStack() as ctx:
        _emit(ctx, tc, t)
    nc.compile()
    _NC_CACHE[key] = nc
    return nc


def make_in_maps(x, position_ids, Wqkv, Wproj):
    x = np.asarray(x, dtype=np.float32)
    pos = np.asarray(position_ids, dtype=np.float64)
    Wqkv = np.asarray(Wqkv, dtype=np.float32)
    Wproj = np.asarray(Wproj, dtype=np.float32)
    bf = ml_dtypes.bfloat16

    inv_freq = 1.0 / (
        ROPE_BASE ** (np.arange(0, HD, 2, dtype=np.float32) / HD)
    )  # [64]
    tri = (np.arange(P)[None, :] >= np.arange(P)[:, None]).astype(bf)

    in_maps = []
    for c in range(8):
        b, g = divmod(c, G)
        freqs = pos[b].astype(np.float32)[:, None] * inv_freq[None, :]  # [N, 64]
        emb = np.concatenate([freqs, freqs], axis=-1)  # [N, 128]
        cosT = np.ascontiguousarray(np.cos(emb).T)  # [128, N]
        sinT = np.ascontiguousarray(np.sin(emb).T)
        sinT[:64] = -sinT[:64]
        wp4 = np.ascontiguousarray(
            Wproj[512 * g : 512 * (g + 1), :]
            .reshape(HPG, P, N)
            .transpose(1, 0, 2)
        ).astype(bf)
        in_maps.append(
            {
                "xT": np.ascontiguousarray(x[b].T).astype(bf),
                "wq": np.ascontiguousarray(
                    Wqkv[:, 512 * g : 512 * (g + 1)]
                ).astype(bf),
                "wk": np.ascontiguousarray(
                    Wqkv[:, 2048 + 512 * g : 2048 + 512 * (g + 1)]
                ).astype(bf),
                "wv": np.ascontiguousarray(
                    Wqkv[:, 4096 + 512 * g : 4096 + 512 * (g + 1)]
                ).astype(bf),
                "wp4": wp4,
                "cosT": cosT,
                "sinT": sinT,
                "tri": tri,
            }
        )
    return in_maps


def kernel(x, position_ids, Wqkv, Wproj, _trace=False, _tmpdir=None):
    nc = build_nc()
    in_maps = make_in_maps(x, position_ids, Wqkv, Wproj)
    res = bass_utils.run_bass_kernel_spmd(
        nc, in_maps, core_ids=list(range(8)), trace=_trace, tmpdir=_tmpdir
    )
    out = np.empty((B, N, C), dtype=np.float32)
    for b in range(B):
        acc = res.results[4 * b]["projB"].astype(np.float32)
        for g in range(1, G):
            acc += res.results[4 * b + g]["projB"].astype(np.float32)
        out[b] = acc
    kernel.last_exec_time_ns = res.exec_time_ns
    kernel.last_results = res
    return out


# revision 21
# speedup vs baseline: 1.0481x; 1.0481x over previous
"""Causal attention block (QKV proj + RoPE + causal SDPA + out proj) on 8
Trainium2 NeuronCores.

Sharding: core c = 4*b + g handles batch b (of 2) and head group g (of 4,
4 heads each).  Each core computes q/k/v for its 4 heads from x[b] and the
matching Wqkv column slices, runs causal SDPA, and contracts its 512
output-feature rows of Wproj, producing a partial projB [2048(tok),
2048(oc)].  The host sums the 4 partials per batch.

v2 design notes (vs the fp32r baseline):
  * All matmul operands are bf16 (PSUM accumulation stays fp32).  bf16
    stationaries enable Fast Weight Load (2 cols/cycle) -- fp32r LDWEIGHTS
    measured ~190ns/tile and made QKV LDW-port-bound (~224ns/MM observed vs
    160ns stream-ideal).  End-to-end bf16 error measured 3.9e-3 <= 2e-2.
  * Softmax row sums are fused into the attn@v matmuls: e-tiles are the
    STATIONARY operand ([128 keys, 128 queries] chunks) and the moving
    operand is v with an appended all-ones column [128 keys, 129].  The
    PSUM result is [queries, hd | rowsum], so the denominator lands as a
    per-partition scalar: reciprocal on [128,1] + tensor_scalar broadcast.
    This removes the separate all-ones rowsum matmul (1/3 of attention
    matmul rows in the baseline).
  * The [q, hd] attention output is transposed back to [hd, q] for the
    projection with SBUF->SBUF dma_start_transpose (xbar), costing no
    engine time.
  * Projection emits projB[tok, oc] (stationary = outT chunk, moving = Wproj
    rows), so neither device nor host transposes the output; output is bf16
    (halves the output DMA).
  * exp runs once per (panel, jb) over both heads' scores ([128, 2, 512-n0]
    strided PSUM read) halving ACT instruction overheads.
  * RoPE reads q/k PSUM directly (swap-halves via partition-offset ACT
    copies + in-place DVE muls); q/k/v PSUM banks free early so the next
    panel's matmuls are never blocked on the RoPE chain.
"""

import sys

if "/opt/trn_rl_repo" not in sys.path:
    sys.path.insert(0, "/opt/trn_rl_repo")

from contextlib import ExitStack

import ml_dtypes
import numpy as np

import concourse.bass as bass  # noqa: F401
import concourse.tile as tile
from concourse import bacc, bass_utils, mybir

F32 = mybir.dt.float32
BF16 = mybir.dt.bfloat16
EXP = mybir.ActivationFunctionType.Exp

B, N, C = 2, 2048, 2048
H = 16  # total heads
HD = C // H  # 128
G = 4  # head groups (cores per batch)
HPG = H // G  # 4 heads per group
P = 128
PANEL = 512
NP = N // PANEL  # 4 token panels
KB = C // P  # 16 contraction blocks
NJB = N // P  # 16 key blocks
SCALE = float(HD) ** -0.5
ROPE_BASE = 10000.0

_NC_CACHE = {}
DEBUG = False


def _bc2(ap, n=2):
    """Broadcast a [128, F] AP across an inserted middle dim -> [128, n, F]."""
    p, f = ap.shape
    return ap.rearrange("p (o n) -> p o n", o=1).broadcast_to([p, n, f])


def _emit(ctx, tc, t):
    nc = tc.nc
    vec = nc.vector
    sca = nc.scalar
    mm = nc.tensor.matmul

    const = ctx.enter_context(tc.tile_pool(name="const", bufs=1))
    wpool = ctx.enter_context(tc.tile_pool(name="w", bufs=2))
    xpool = ctx.enter_context(tc.tile_pool(name="x", bufs=2))
    qkpool = ctx.enter_context(tc.tile_pool(name="qk", bufs=2))
    vpool = ctx.enter_context(tc.tile_pool(name="v", bufs=2))
    rpool = ctx.enter_context(tc.tile_pool(name="rope", bufs=2))
    epool = ctx.enter_context(tc.tile_pool(name="e", bufs=4))
    opool = ctx.enter_context(tc.tile_pool(name="o", bufs=1))
    pnpool = ctx.enter_context(tc.tile_pool(name="pn", bufs=2))
    rspool = ctx.enter_context(tc.tile_pool(name="rs", bufs=4))
    poutp = ctx.enter_context(tc.tile_pool(name="pout", bufs=4))
    ps = ctx.enter_context(tc.tile_pool(name="ps", bufs=1, space="PSUM"))

    cosT = const.tile([P, N], F32)
    sinT = const.tile([P, N], F32)
    tri = const.tile([P, P], BF16)
    ident = const.tile([P, P], BF16)
    zeros = const.tile([P, 264], BF16)
    # consts go on the gpsimd DMA queue so they don't delay the first
    # weight/x loads on the sync queue
    nc.gpsimd.dma_start(cosT, t["cosT"])
    nc.gpsimd.dma_start(sinT, t["sinT"])
    nc.gpsimd.dma_start(tri, t["tri"])
    nc.gpsimd.dma_start(ident, t["ident"])
    vec.memset(zeros, 0.0)

    # warm up the PE clock (HAM un-throttles after ~3.4us of activity)
    # while the first weight/x DMAs are in flight
    warm_ps = ps.tile([P, P], F32, tag="PO0", name="warm_ps")
    for _ in range(32):
        mm(warm_ps, zeros[:, 0:128], zeros[:, 0:128], skip_group_check=True)

    # wp4 loaded later (during sweep 0) to keep the startup DMA short
    wp4 = const.tile([P, HPG, N], BF16, name="wp4")

    outT = [
        opool.tile([P, N], BF16, tag=f"outT{h}", name=f"outT{h}")
        for h in range(HPG)
    ]


    def rope_evac(psrc, which):
        """One fast ACT copy frees the q/k PSUM banks."""
        raw = rpool.tile([P, 2, PANEL], F32, tag=f"raw{which}", name="raw")
        sca.copy(raw, psrc)
        return raw

    def rope_finish(raw, dst, sl):
        """dst[:, :, sl] = raw*cos + swap64(raw)*sin'  (sin' pre-negated on
        partitions 0-63)."""
        rw = rpool.tile([P, 2, PANEL], F32, tag="rw", name="rw")
        sca.copy(rw[0:64], raw[64:128])
        sca.copy(rw[64:128], raw[0:64])
        vec.tensor_mul(rw, rw, _bc2(sinT[:, sl]))
        vec.tensor_mul(raw, raw, _bc2(cosT[:, sl]))
        vec.tensor_add(dst[:, :, sl], raw, rw)

    def load_w(sweep):
        """Per-sweep 256-col slices of wq/wk/wv -> [128, KB, 256] bf16."""
        w_sb = {}
        for wname in ("wq", "wk", "wv"):
            w3 = t[wname][sweep].rearrange("p (kb f) -> p kb f", kb=KB)
            # split into halves so the first matmuls can start early
            w_t = wpool.tile([P, KB, 256], BF16, tag=wname, name=wname)
            for hf in range(2):
                nc.sync.dma_start(
                    w_t[:, 8 * hf : 8 * hf + 8, :],
                    w3[:, 8 * hf : 8 * hf + 8, :],
                )
            w_sb[wname] = w_t
        return w_sb

    for sweep in range(2):
        w_sb = load_w(sweep) if sweep == 0 else w_next  # noqa: F821
        # per-sweep state
        qq = qkpool.tile([P, 2, N], BF16, tag="qq", name="qq")
        kk = qkpool.tile([P, 2, N], BF16, tag="kk", name="kk")
        v_sb = vpool.tile([P, NJB, 2, 132], BF16, tag="v", name="v_sb")
        vec.memset(v_sb[:, :, :, 128:129], 1.0)

        # ---- phase A: QKV + RoPE for this sweep's 2 heads ----
        for p in range(NP):
            sl = slice(PANEL * p, PANEL * (p + 1))
            pq01 = ps.tile([P, 2, PANEL], F32, tag="SC0", name="pq01")
            pk01 = ps.tile([P, 2, PANEL], F32, tag="SC1", name="pk01")
            pv = [
                ps.tile([P, 256], F32, tag=f"PO{tb}", name=f"pv{tb}")
                for tb in range(4)
            ]
            xt = [None, None]
            for hb in range(2):
                xt[hb] = xpool.tile([P, KB // 2, PANEL], BF16, tag="x", name="xt")
                xsrc = t["xTile"][2 * p + hb].rearrange(
                    "q (kb n) -> q kb n", kb=KB // 2
                )
                (nc.scalar if hb == 0 else nc.sync).dma_start(xt[hb], xsrc)
            # q,k,v grouped per xt half: q/k PSUM banks drain while the later
            # groups stream, so the next panel never stalls on RoPE; each
            # half only needs its own xt DMA.
            for hb in range(2):
                for wn, pdst in (("wq", pq01), ("wk", pk01)):
                    for kbl in range(KB // 2):
                        kb = 8 * hb + kbl
                        st, sp = kb == 0, kb == KB - 1
                        x_k = xt[hb][:, kbl]
                        mm(pdst[:, 0], w_sb[wn][:, kb, 0:128], x_k, start=st, stop=sp)
                        mm(pdst[:, 1], w_sb[wn][:, kb, 128:256], x_k, start=st, stop=sp)
                    if hb == 1 and wn == "wq":
                        raw_q = rope_evac(pq01, "q")
                for kbl in range(KB // 2):
                    kb = 8 * hb + kbl
                    st, sp = kb == 0, kb == KB - 1
                    x_k = xt[hb][:, kbl]
                    for tb in range(4):
                        mm(
                            pv[tb],
                            x_k[:, 128 * tb : 128 * (tb + 1)],
                            w_sb["wv"][:, kb],
                            start=st,
                            stop=sp,
                        )
                if hb == 1:
                    raw_k = rope_evac(pk01, "k")
            for tb in range(4):
                sca.copy(
                    v_sb[:, 4 * p + tb, :, 0:128],
                    pv[tb].rearrange("p (h f) -> p h f", h=2),
                )
            rope_finish(raw_q, qq, sl)
            rope_finish(raw_k, kk, sl)

        # prefetch next sweep's weights / wp4 during attention
        if sweep == 0:
            w_next = load_w(1)
            nc.sync.dma_start(wp4, t["wp4"])

        # ---- phase B: causal SDPA (+ proj on sweep 1) ----
        for p in range(NP):
            njb = 4 * p + 4
            po = {
                (hh, pair): ps.tile(
                    [P, 2, 132], F32, tag=f"PO{2 * hh + pair}", name="po"
                )
                for hh in range(2)
                for pair in range(2)
            }
            # Two accumulation groups share each po bank, but a start=True
            # matmul clears has_written for the WHOLE bank.  So pre-zero the
            # bank with one dummy matmul (sets has_written everywhere) and
            # accumulate with start=False.  Emitted lazily (just before the
            # first AV matmul) so the PE isn't blocked on the banks at the
            # panel boundary.
            def emit_dummies():
                for key in po:
                    mm(
                        po[key].rearrange("p a b -> p (a b)"),
                        tri,
                        zeros,
                        start=True,
                        stop=False,
                        skip_group_check=True,
                    )

            e_tiles = []

            def emit_av(jj):
                e1, td = e_tiles[jj]
                for hh in range(2):
                    for qc in range(max(0, td), 4):
                        mm(
                            po[hh, qc // 2][:, qc % 2, 0:129],
                            e1[:, hh, 128 * qc : 128 * (qc + 1)],
                            v_sb[:, jj, hh, 0:129],
                            start=False,
                            stop=(jj == 4 * p + qc),
                            skip_group_check=True,
                        )

            po_n = {
                hh: pnpool.tile([P, PANEL], BF16, tag=f"pn{hh}", name="po_n")
                for hh in range(2)
            }

            def normalize(qc):
                # qc's accumulation finished at jj = 4p+qc: divide by the
                # fused rowsum (col 128), immediately, so the po bank frees
                # early.
                for hh in range(2):
                    src = po[hh, qc // 2][:, qc % 2]
                    rs_rec = rspool.tile([P, 1], F32, tag="rs", name="rs_rec")
                    vec.reciprocal(rs_rec, src[:, 128:129])
                    vec.tensor_scalar_mul(
                        po_n[hh][:, 128 * qc : 128 * (qc + 1)], src[:, 0:128], rs_rec
                    )

            def transpose_out():
                # [q, hd] -> [hd, q] via PE transpose-mode (bf16 out is legal
                # for transposes; both heads' 4 chunks fit half a SC slot)
                for hh in range(2):
                    tr = ps.tile([P, 4, P], BF16, tag=f"SC{hh}", name="tr")
                    for qc in range(4):
                        nc.tensor.transpose(
                            tr[:, qc], po_n[hh][:, 128 * qc : 128 * (qc + 1)], ident
                        )
                    (vec.tensor_copy if hh == 0 else sca.copy)(
                        outT[2 * sweep + hh][:, PANEL * p : PANEL * (p + 1)],
                        tr.rearrange("p a b -> p (a b)"),
                    )

            def after_av(jj):
                # Only normalize when a whole po BANK (a qc pair) is done:
                # a DVE read of a bank the PE is still accumulating into is a
                # PSUM bank collision (address-level dep tracking won't order
                # it).  Odd qc first: its reciprocal waits on the pair's last
                # matmul, and DVE's in-order FIFO then protects the even qc.
                qc = jj - 4 * p
                if qc in (1, 3):
                    normalize(qc)
                    normalize(qc - 1)

            for jj in range(njb):
                td = jj - 4 * p  # diagonal sub-block index if >= 0
                n0 = 128 * td if td > 0 else 0
                sc = ps.tile([P, 2, PANEL], F32, tag=f"SC{jj % 2}", name="sc")
                for hh in range(2):
                    mm(
                        sc[:, hh, n0:],
                        kk[:, hh, 128 * jj : 128 * (jj + 1)],
                        qq[:, hh, PANEL * p + n0 : PANEL * (p + 1)],
                    )
                e1 = epool.tile([P, 2, PANEL], BF16, tag="e1", name="e1")
                sca.activation(e1[:, :, n0:], sc[:, :, n0:], EXP, scale=SCALE)
                if td >= 0:
                    dsl = slice(128 * td, 128 * (td + 1))
                    vec.tensor_mul(e1[:, :, dsl], e1[:, :, dsl], _bc2(tri))
                e_tiles.append((e1, td))
                if jj == 2:
                    emit_dummies()
                if jj >= 2:
                    emit_av(jj - 2)
                    after_av(jj - 2)
            emit_av(njb - 2)
            after_av(njb - 2)
            emit_av(njb - 1)
            after_av(njb - 1)
            transpose_out()

            if sweep == 1:
                # proj for this panel: outT[0..3][:, psl] are final now
                for tc in range(4):
                    tsl = slice(PANEL * p + 128 * tc, PANEL * p + 128 * (tc + 1))
                    for occ in range(4):
                        pj = ps.tile(
                            [P, PANEL], F32, tag=f"SC{occ % 2}", name="pj"
                        )
                        osl = slice(PANEL * occ, PANEL * (occ + 1))
                        for h in range(HPG):
                            mm(
                                pj,
                                outT[h][:, tsl],
                                wp4[:, h, osl],
                                start=(h == 0),
                                stop=(h == HPG - 1),
                            )
                        o_t = poutp.tile([P, PANEL], BF16, tag="pout", name="o_t")
                        if occ % 2 == 0:
                            sca.copy(o_t, pj)
                        else:
                            vec.tensor_copy(o_t, pj)
                        (nc.sync if occ % 2 else nc.scalar).dma_start(
                            t["projB"][tsl, osl], o_t
                        )

    if DEBUG:
        for h in range(HPG):
            nc.sync.dma_start(t[f"dbg_o{h}"], outT[h])


def build_nc():
    key = (DEBUG,)
    if key in _NC_CACHE:
        return _NC_CACHE[key]
    nc = bacc.Bacc("TRN2", target_bir_lowering=False, debug=False)
    t = {}
    t["xTile"] = nc.dram_tensor(
        "xTile", [2 * NP, P, (KB // 2) * PANEL], BF16, kind="ExternalInput"
    ).ap()
    t["wq"] = nc.dram_tensor("wq", [2, P, KB * 256], BF16, kind="ExternalInput").ap()
    t["wk"] = nc.dram_tensor("wk", [2, P, KB * 256], BF16, kind="ExternalInput").ap()
    t["wv"] = nc.dram_tensor("wv", [2, P, KB * 256], BF16, kind="ExternalInput").ap()
    t["wp4"] = nc.dram_tensor("wp4", [P, HPG, N], BF16, kind="ExternalInput").ap()
    t["cosT"] = nc.dram_tensor("cosT", [P, N], F32, kind="ExternalInput").ap()
    t["sinT"] = nc.dram_tensor("sinT", [P, N], F32, kind="ExternalInput").ap()
    t["tri"] = nc.dram_tensor("tri", [P, P], BF16, kind="ExternalInput").ap()
    t["ident"] = nc.dram_tensor("ident", [P, P], BF16, kind="ExternalInput").ap()
    t["projB"] = nc.dram_tensor("projB", [N, N], BF16, kind="ExternalOutput").ap()
    if DEBUG:
        for h in range(HPG):
            t[f"dbg_o{h}"] = nc.dram_tensor(
                f"dbg_o{h}", [P, N], BF16, kind="ExternalOutput"
            ).ap()
    with tile.TileContext(nc) as tc, ExitStack() as ctx:
        _emit(ctx, tc, t)
    nc.compile()
    _NC_CACHE[key] = nc
    return nc


def make_in_maps(x, position_ids, Wqkv, Wproj):
    x = np.asarray(x, dtype=np.float32)
    pos = np.asarray(position_ids, dtype=np.float64)
    Wqkv = np.asarray(Wqkv, dtype=np.float32)
    Wproj = np.asarray(Wproj, dtype=np.float32)
    bf = ml_dtypes.bfloat16

    inv_freq = 1.0 / (
        ROPE_BASE ** (np.arange(0, HD, 2, dtype=np.float32) / HD)
    )  # [64]
    tri = (np.arange(P)[None, :] >= np.arange(P)[:, None]).astype(bf)
    ident = np.eye(P, dtype=np.float32).astype(bf)

    def xtile(xb):
        # [p, hb, q, kbl, t] tiling of x^T so each [128, 8*512] DMA is
        # contiguous per partition
        xT = np.ascontiguousarray(xb.T)  # [C, N]
        v = xT.reshape(2, 8, P, NP, PANEL).transpose(3, 0, 2, 1, 4)
        return np.ascontiguousarray(v.reshape(2 * NP, P, 8 * PANEL)).astype(bf)

    def wtile(w):
        # [sweep, p, kb, f] tiling of a [C, 512] weight slice
        v = w.reshape(KB, P, 2, 256).transpose(2, 1, 0, 3)
        return np.ascontiguousarray(v.reshape(2, P, KB * 256)).astype(bf)

    in_maps = []
    for c in range(8):
        b, g = divmod(c, G)
        freqs = pos[b].astype(np.float32)[:, None] * inv_freq[None, :]  # [N, 64]
        emb = np.concatenate([freqs, freqs], axis=-1)  # [N, 128]
        cosT = np.ascontiguousarray(np.cos(emb).T)  # [128, N]
        sinT = np.ascontiguousarray(np.sin(emb).T)
        sinT[:64] = -sinT[:64]
        wp4 = np.ascontiguousarray(
            Wproj[512 * g : 512 * (g + 1), :]
            .reshape(HPG, P, N)
            .transpose(1, 0, 2)
        ).astype(bf)
        in_maps.append(
            {
                "xTile": xtile(x[b]),
                "wq": wtile(Wqkv[:, 512 * g : 512 * (g + 1)]),
                "wk": wtile(Wqkv[:, 2048 + 512 * g : 2048 + 512 * (g + 1)]),
                "wv": wtile(Wqkv[:, 4096 + 512 * g : 4096 + 512 * (g + 1)]),
                "wp4": wp4,
                "cosT": cosT,
                "sinT": sinT,
                "tri": tri,
                "ident": ident,
            }
        )
    return in_maps


def kernel(x, position_ids, Wqkv, Wproj, _trace=False, _tmpdir=None):
    nc = build_nc()
    in_maps = make_in_maps(x, position_ids, Wqkv, Wproj)
    res = bass_utils.run_bass_kernel_spmd(
        nc, in_maps, core_ids=list(range(8)), trace=_trace, tmpdir=_tmpdir
    )
    out = np.empty((B, N, C), dtype=np.float32)
    for b in range(B):
        acc = res.results[4 * b]["projB"].astype(np.float32)
        for g in range(1, G):
            acc += res.results[4 * b + g]["projB"].astype(np.float32)
        out[b] = acc
    kernel.last_exec_time_ns = res.exec_time_ns
    kernel.last_results = res
    return out


# revision 24
# speedup vs baseline: 1.2404x; 1.1835x over previous
"""Causal attention block (QKV proj + RoPE + causal SDPA + out proj) on 8
Trainium2 NeuronCores.

Sharding: core c = 4*b + g handles batch b (of 2) and head group g (of 4,
4 heads each).  Each core computes q/k/v for its 4 heads from x[b] and the
matching Wqkv column slices, runs causal SDPA, and contracts its 512
output-feature rows of Wproj, producing a partial projB [2048(tok),
2048(oc)].  The host sums the 4 partials per batch.

v2 design notes (vs the fp32r baseline):
  * All matmul operands are bf16 (PSUM accumulation stays fp32).  bf16
    stationaries enable Fast Weight Load (2 cols/cycle) -- fp32r LDWEIGHTS
    measured ~190ns/tile and made QKV LDW-port-bound (~224ns/MM observed vs
    160ns stream-ideal).  End-to-end bf16 error measured 3.9e-3 <= 2e-2.
  * Softmax row sums are fused into the attn@v matmuls: e-tiles are the
    STATIONARY operand ([128 keys, 128 queries] chunks) and the moving
    operand is v with an appended all-ones column [128 keys, 129].  The
    PSUM result is [queries, hd | rowsum], so the denominator lands as a
    per-partition scalar: reciprocal on [128,1] + tensor_scalar broadcast.
    This removes the separate all-ones rowsum matmul (1/3 of attention
    matmul rows in the baseline).
  * The [q, hd] attention output is transposed back to [hd, q] for the
    projection with SBUF->SBUF dma_start_transpose (xbar), costing no
    engine time.
  * Projection emits projB[tok, oc] (stationary = outT chunk, moving = Wproj
    rows), so neither device nor host transposes the output; output is bf16
    (halves the output DMA).
  * exp runs once per (panel, jb) over both heads' scores ([128, 2, 512-n0]
    strided PSUM read) halving ACT instruction overheads.
  * RoPE reads q/k PSUM directly (swap-halves via partition-offset ACT
    copies + in-place DVE muls); q/k/v PSUM banks free early so the next
    panel's matmuls are never blocked on the RoPE chain.
"""

import sys

if "/opt/trn_rl_repo" not in sys.path:
    sys.path.insert(0, "/opt/trn_rl_repo")

from contextlib import ExitStack

import ml_dtypes
import numpy as np

import concourse.bass as bass  # noqa: F401
import concourse.tile as tile
from concourse import bacc, bass_utils, mybir

F32 = mybir.dt.float32
BF16 = mybir.dt.bfloat16
EXP = mybir.ActivationFunctionType.Exp

B, N, C = 2, 2048, 2048
H = 16  # total heads
HD = C // H  # 128
G = 4  # head groups (cores per batch)
HPG = H // G  # 4 heads per group
P = 128
PANEL = 512
NP = N // PANEL  # 4 token panels
KB = C // P  # 16 contraction blocks
NJB = N // P  # 16 key blocks
SCALE = float(HD) ** -0.5
ROPE_BASE = 10000.0

_NC_CACHE = {}
DEBUG = False


def _bc2(ap, n=2):
    """Broadcast a [128, F] AP across an inserted middle dim -> [128, n, F]."""
    p, f = ap.shape
    return ap.rearrange("p (o n) -> p o n", o=1).broadcast_to([p, n, f])


def _emit(ctx, tc, t):
    nc = tc.nc
    vec = nc.vector
    sca = nc.scalar
    mm = nc.tensor.matmul

    const = ctx.enter_context(tc.tile_pool(name="const", bufs=1))
    wpool = ctx.enter_context(tc.tile_pool(name="w", bufs=2))
    xpool = ctx.enter_context(tc.tile_pool(name="x", bufs=2))
    qkpool = ctx.enter_context(tc.tile_pool(name="qk", bufs=2))
    vpool = ctx.enter_context(tc.tile_pool(name="v", bufs=2))
    rpool = ctx.enter_context(tc.tile_pool(name="rope", bufs=2))
    epool = ctx.enter_context(tc.tile_pool(name="e", bufs=4))
    opool = ctx.enter_context(tc.tile_pool(name="o", bufs=1))
    poutp = ctx.enter_context(tc.tile_pool(name="pout", bufs=4))
    ps = ctx.enter_context(tc.tile_pool(name="ps", bufs=1, space="PSUM"))

    cosT = const.tile([P, N], F32)
    sinT = const.tile([P, N], F32)
    tri = const.tile([P, P], BF16)
    ones = const.tile([P, P], BF16)
    warmz = const.tile([P, P], BF16)
    # consts go on the gpsimd DMA queue so they don't delay the first
    # weight/x loads on the sync queue
    nc.gpsimd.dma_start(cosT, t["cosT"])
    nc.gpsimd.dma_start(sinT, t["sinT"])
    nc.gpsimd.dma_start(tri, t["tri"])
    vec.memset(ones, 1.0)
    vec.memset(warmz, 0.0)

    # warm up the PE clock (HAM un-throttles after ~3.4us of activity)
    # while the first weight/x DMAs are in flight
    warm_ps = ps.tile([P, P], F32, tag="PO0", name="warm_ps")
    for _ in range(32):
        mm(warm_ps, warmz, warmz, skip_group_check=True)

    # wp4 loaded later (during sweep 0) to keep the startup DMA short
    wp4 = const.tile([P, HPG, N], BF16, name="wp4")

    outT = [
        opool.tile([P, N], BF16, tag=f"outT{h}", name=f"outT{h}")
        for h in range(HPG)
    ]


    def rope_evac(psrc, which):
        """One fast ACT copy frees the q/k PSUM banks."""
        raw = rpool.tile([P, 2, PANEL], F32, tag=f"raw{which}", name="raw")
        sca.copy(raw, psrc)
        return raw

    def rope_finish(raw, dst, sl):
        """dst[:, :, sl] = raw*cos + swap64(raw)*sin'  (sin' pre-negated on
        partitions 0-63)."""
        rw = rpool.tile([P, 2, PANEL], F32, tag="rw", name="rw")
        sca.copy(rw[0:64], raw[64:128])
        sca.copy(rw[64:128], raw[0:64])
        vec.tensor_mul(rw, rw, _bc2(sinT[:, sl]))
        vec.tensor_mul(raw, raw, _bc2(cosT[:, sl]))
        vec.tensor_add(dst[:, :, sl], raw, rw)

    def load_w(sweep):
        """Per-sweep 256-col slices of wq/wk/wv -> [128, KB, 256] bf16."""
        w_sb = {}
        for wname in ("wq", "wk", "wv"):
            w3 = t[wname][sweep].rearrange("p (kb f) -> p kb f", kb=KB)
            # split into halves so the first matmuls can start early
            w_t = wpool.tile([P, KB, 256], BF16, tag=wname, name=wname)
            for hf in range(2):
                nc.sync.dma_start(
                    w_t[:, 8 * hf : 8 * hf + 8, :],
                    w3[:, 8 * hf : 8 * hf + 8, :],
                )
            w_sb[wname] = w_t
        return w_sb

    for sweep in range(2):
        w_sb = load_w(sweep) if sweep == 0 else w_next  # noqa: F821
        # per-sweep state
        qq = qkpool.tile([P, 2, N], BF16, tag="qq", name="qq")
        kk = qkpool.tile([P, 2, N], BF16, tag="kk", name="kk")
        v_sb = vpool.tile([P, NJB, 2, P], BF16, tag="v", name="v_sb")

        # ---- phase A: QKV + RoPE for this sweep's 2 heads ----
        for p in range(NP):
            sl = slice(PANEL * p, PANEL * (p + 1))
            pq01 = ps.tile([P, 2, PANEL], F32, tag="SC0", name="pq01")
            pk01 = ps.tile([P, 2, PANEL], F32, tag="SC1", name="pk01")
            pv = [
                ps.tile([P, 256], F32, tag=f"PO{tb}", name=f"pv{tb}")
                for tb in range(4)
            ]
            xt = [None, None]
            for hb in range(2):
                xt[hb] = xpool.tile([P, KB // 2, PANEL], BF16, tag="x", name="xt")
                xsrc = t["xTile"][2 * p + hb].rearrange(
                    "q (kb n) -> q kb n", kb=KB // 2
                )
                (nc.scalar if hb == 0 else nc.sync).dma_start(xt[hb], xsrc)
            # q,k,v grouped per xt half: q/k PSUM banks drain while the later
            # groups stream, so the next panel never stalls on RoPE; each
            # half only needs its own xt DMA.
            for hb in range(2):
                for wn, pdst in (("wq", pq01), ("wk", pk01)):
                    for kbl in range(KB // 2):
                        kb = 8 * hb + kbl
                        st, sp = kb == 0, kb == KB - 1
                        x_k = xt[hb][:, kbl]
                        mm(pdst[:, 0], w_sb[wn][:, kb, 0:128], x_k, start=st, stop=sp)
                        mm(pdst[:, 1], w_sb[wn][:, kb, 128:256], x_k, start=st, stop=sp)
                    if hb == 1 and wn == "wq":
                        raw_q = rope_evac(pq01, "q")
                for kbl in range(KB // 2):
                    kb = 8 * hb + kbl
                    st, sp = kb == 0, kb == KB - 1
                    x_k = xt[hb][:, kbl]
                    for tb in range(4):
                        mm(
                            pv[tb],
                            x_k[:, 128 * tb : 128 * (tb + 1)],
                            w_sb["wv"][:, kb],
                            start=st,
                            stop=sp,
                        )
                if hb == 1:
                    raw_k = rope_evac(pk01, "k")
            for tb in range(4):
                sca.copy(
                    v_sb[:, 4 * p + tb],
                    pv[tb].rearrange("p (h f) -> p h f", h=2),
                )
            rope_finish(raw_q, qq, sl)
            rope_finish(raw_k, kk, sl)

        # prefetch next sweep's weights / wp4 during attention
        if sweep == 0:
            w_next = load_w(1)
            nc.sync.dma_start(wp4, t["wp4"])

        # ---- phase B: causal SDPA (+ proj on sweep 1) ----
        for p in range(NP):
            njb = 4 * p + 4
            sl = slice(PANEL * p, PANEL * (p + 1))
            po = {
                hh: ps.tile([P, PANEL], F32, tag=f"PO{hh}", name="po")
                for hh in range(2)
            }
            prs = {
                hh: ps.tile([P, PANEL], F32, tag=f"PO{2 + hh}", name="prs")
                for hh in range(2)
            }
            e_tiles = []

            def emit_av(jj):
                # attn@v (stationary = v chunk, long moving stream) plus the
                # rowsum matmul against all-ones; both accumulate over jj.
                e1, n0 = e_tiles[jj]
                st, sp = jj == 0, jj == njb - 1
                for hh in range(2):
                    mm(
                        po[hh][:, n0:],
                        v_sb[:, jj, hh],
                        e1[:, hh, n0:],
                        start=st,
                        stop=sp,
                    )
                    mm(
                        prs[hh][:, n0:], ones, e1[:, hh, n0:], start=st, stop=sp
                    )

            for jj in range(njb):
                td = jj - 4 * p  # diagonal sub-block index if >= 0
                n0 = 128 * td if td > 0 else 0
                sc = ps.tile([P, 2, PANEL], F32, tag=f"SC{jj % 2}", name="sc")
                for hh in range(2):
                    mm(
                        sc[:, hh, n0:],
                        kk[:, hh, 128 * jj : 128 * (jj + 1)],
                        qq[:, hh, PANEL * p + n0 : PANEL * (p + 1)],
                    )
                e1 = epool.tile([P, 2, PANEL], BF16, tag="e1", name="e1")
                sca.activation(e1[:, :, n0:], sc[:, :, n0:], EXP, scale=SCALE)
                if td >= 0:
                    dsl = slice(128 * td, 128 * (td + 1))
                    vec.tensor_mul(e1[:, :, dsl], e1[:, :, dsl], _bc2(tri))
                e_tiles.append((e1, n0))
                if jj >= 2:
                    emit_av(jj - 2)
            emit_av(njb - 2)
            emit_av(njb - 1)

            # outT = po / rowsum, straight into [hd, tok] layout
            for hh in range(2):
                rcp = rpool.tile([P, PANEL], F32, tag="rcp", name="rcp")
                vec.reciprocal_approx_fast(rcp, prs[hh])
                vec.tensor_mul(outT[2 * sweep + hh][:, sl], po[hh], rcp)

            if sweep == 1:
                # proj for this panel: outT[0..3][:, psl] are final now
                for tc in range(4):
                    tsl = slice(PANEL * p + 128 * tc, PANEL * p + 128 * (tc + 1))
                    for occ in range(4):
                        pj = ps.tile(
                            [P, PANEL], F32, tag=f"SC{occ % 2}", name="pj"
                        )
                        osl = slice(PANEL * occ, PANEL * (occ + 1))
                        for h in range(HPG):
                            mm(
                                pj,
                                outT[h][:, tsl],
                                wp4[:, h, osl],
                                start=(h == 0),
                                stop=(h == HPG - 1),
                            )
                        o_t = poutp.tile([P, PANEL], BF16, tag="pout", name="o_t")
                        if occ % 2 == 0:
                            sca.copy(o_t, pj)
                        else:
                            vec.tensor_copy(o_t, pj)
                        (nc.sync if occ % 2 else nc.scalar).dma_start(
                            t["projB"][tsl, osl], o_t
                        )

    if DEBUG:
        for h in range(HPG):
            nc.sync.dma_start(t[f"dbg_o{h}"], outT[h])


def build_nc():
    key = (DEBUG,)
    if key in _NC_CACHE:
        return _NC_CACHE[key]
    nc = bacc.Bacc("TRN2", target_bir_lowering=False, debug=False)
    t = {}
    t["xTile"] = nc.dram_tensor(
        "xTile", [2 * NP, P, (KB // 2) * PANEL], BF16, kind="ExternalInput"
    ).ap()
    t["wq"] = nc.dram_tensor("wq", [2, P, KB * 256], BF16, kind="ExternalInput").ap()
    t["wk"] = nc.dram_tensor("wk", [2, P, KB * 256], BF16, kind="ExternalInput").ap()
    t["wv"] = nc.dram_tensor("wv", [2, P, KB * 256], BF16, kind="ExternalInput").ap()
    t["wp4"] = nc.dram_tensor("wp4", [P, HPG, N], BF16, kind="ExternalInput").ap()
    t["cosT"] = nc.dram_tensor("cosT", [P, N], F32, kind="ExternalInput").ap()
    t["sinT"] = nc.dram_tensor("sinT", [P, N], F32, kind="ExternalInput").ap()
    t["tri"] = nc.dram_tensor("tri", [P, P], BF16, kind="ExternalInput").ap()
    t["projB"] = nc.dram_tensor("projB", [N, N], BF16, kind="ExternalOutput").ap()
    if DEBUG:
        for h in range(HPG):
            t[f"dbg_o{h}"] = nc.dram_tensor(
                f"dbg_o{h}", [P, N], BF16, kind="ExternalOutput"
            ).ap()
    with tile.TileContext(nc) as tc, ExitStack() as ctx:
        _emit(ctx, tc, t)
    nc.compile()
    _NC_CACHE[key] = nc
    return nc


def make_in_maps(x, position_ids, Wqkv, Wproj):
    x = np.asarray(x, dtype=np.float32)
    pos = np.asarray(position_ids, dtype=np.float64)
    Wqkv = np.asarray(Wqkv, dtype=np.float32)
    Wproj = np.asarray(Wproj, dtype=np.float32)
    bf = ml_dtypes.bfloat16

    inv_freq = 1.0 / (
        ROPE_BASE ** (np.arange(0, HD, 2, dtype=np.float32) / HD)
    )  # [64]
    tri = (np.arange(P)[None, :] >= np.arange(P)[:, None]).astype(bf)

    def xtile(xb):
        # [p, hb, q, kbl, t] tiling of x^T so each [128, 8*512] DMA is
        # contiguous per partition
        xT = np.ascontiguousarray(xb.T)  # [C, N]
        v = xT.reshape(2, 8, P, NP, PANEL).transpose(3, 0, 2, 1, 4)
        return np.ascontiguousarray(v.reshape(2 * NP, P, 8 * PANEL)).astype(bf)

    def wtile(w):
        # [sweep, p, kb, f] tiling of a [C, 512] weight slice
        v = w.reshape(KB, P, 2, 256).transpose(2, 1, 0, 3)
        return np.ascontiguousarray(v.reshape(2, P, KB * 256)).astype(bf)

    in_maps = []
    for c in range(8):
        b, g = divmod(c, G)
        freqs = pos[b].astype(np.float32)[:, None] * inv_freq[None, :]  # [N, 64]
        emb = np.concatenate([freqs, freqs], axis=-1)  # [N, 128]
        cosT = np.ascontiguousarray(np.cos(emb).T)  # [128, N]
        sinT = np.ascontiguousarray(np.sin(emb).T)
        sinT[:64] = -sinT[:64]
        wp4 = np.ascontiguousarray(
            Wproj[512 * g : 512 * (g + 1), :]
            .reshape(HPG, P, N)
            .transpose(1, 0, 2)
        ).astype(bf)
        in_maps.append(
            {
                "xTile": xtile(x[b]),
                "wq": wtile(Wqkv[:, 512 * g : 512 * (g + 1)]),
                "wk": wtile(Wqkv[:, 2048 + 512 * g : 2048 + 512 * (g + 1)]),
                "wv": wtile(Wqkv[:, 4096 + 512 * g : 4096 + 512 * (g + 1)]),
                "wp4": wp4,
                "cosT": cosT,
                "sinT": sinT,
                "tri": tri,
            }
        )
    return in_maps


def kernel(x, position_ids, Wqkv, Wproj, _trace=False, _tmpdir=None):
    nc = build_nc()
    in_maps = make_in_maps(x, position_ids, Wqkv, Wproj)
    res = bass_utils.run_bass_kernel_spmd(
        nc, in_maps, core_ids=list(range(8)), trace=_trace, tmpdir=_tmpdir
    )
    out = np.empty((B, N, C), dtype=np.float32)
    for b in range(B):
        acc = res.results[4 * b]["projB"].astype(np.float32)
        for g in range(1, G):
            acc += res.results[4 * b + g]["projB"].astype(np.float32)
        out[b] = acc
    kernel.last_exec_time_ns = res.exec_time_ns
    kernel.last_results = res
    return out


# revision 27
# speedup vs baseline: 1.2444x; 1.0032x over previous
"""Causal attention block (QKV proj + RoPE + causal SDPA + out proj) on 8
Trainium2 NeuronCores.

Sharding: core c = 4*b + g handles batch b (of 2) and head group g (of 4,
4 heads each).  Each core computes q/k/v for its 4 heads from x[b] and the
matching Wqkv column slices, runs causal SDPA, and contracts its 512
output-feature rows of Wproj, producing a partial projB [2048(tok),
2048(oc)].  The host sums the 4 partials per batch.

v2 design notes (vs the fp32r baseline):
  * All matmul operands are bf16 (PSUM accumulation stays fp32).  bf16
    stationaries enable Fast Weight Load (2 cols/cycle) -- fp32r LDWEIGHTS
    measured ~190ns/tile and made QKV LDW-port-bound (~224ns/MM observed vs
    160ns stream-ideal).  End-to-end bf16 error measured 3.9e-3 <= 2e-2.
  * Softmax row sums are fused into the attn@v matmuls: e-tiles are the
    STATIONARY operand ([128 keys, 128 queries] chunks) and the moving
    operand is v with an appended all-ones column [128 keys, 129].  The
    PSUM result is [queries, hd | rowsum], so the denominator lands as a
    per-partition scalar: reciprocal on [128,1] + tensor_scalar broadcast.
    This removes the separate all-ones rowsum matmul (1/3 of attention
    matmul rows in the baseline).
  * The [q, hd] attention output is transposed back to [hd, q] for the
    projection with SBUF->SBUF dma_start_transpose (xbar), costing no
    engine time.
  * Projection emits projB[tok, oc] (stationary = outT chunk, moving = Wproj
    rows), so neither device nor host transposes the output; output is bf16
    (halves the output DMA).
  * exp runs once per (panel, jb) over both heads' scores ([128, 2, 512-n0]
    strided PSUM read) halving ACT instruction overheads.
  * RoPE reads q/k PSUM directly (swap-halves via partition-offset ACT
    copies + in-place DVE muls); q/k/v PSUM banks free early so the next
    panel's matmuls are never blocked on the RoPE chain.
"""

import sys

if "/opt/trn_rl_repo" not in sys.path:
    sys.path.insert(0, "/opt/trn_rl_repo")

from contextlib import ExitStack

import ml_dtypes
import numpy as np

import concourse.bass as bass  # noqa: F401
import concourse.tile as tile
from concourse import bacc, bass_utils, mybir

F32 = mybir.dt.float32
BF16 = mybir.dt.bfloat16
EXP = mybir.ActivationFunctionType.Exp

B, N, C = 2, 2048, 2048
H = 16  # total heads
HD = C // H  # 128
G = 4  # head groups (cores per batch)
HPG = H // G  # 4 heads per group
P = 128
PANEL = 512
NP = N // PANEL  # 4 token panels
KB = C // P  # 16 contraction blocks
NJB = N // P  # 16 key blocks
SCALE = float(HD) ** -0.5
ROPE_BASE = 10000.0

_NC_CACHE = {}
DEBUG = False


def _bc2(ap, n=2):
    """Broadcast a [128, F] AP across an inserted middle dim -> [128, n, F]."""
    p, f = ap.shape
    return ap.rearrange("p (o n) -> p o n", o=1).broadcast_to([p, n, f])


def _emit(ctx, tc, t):
    nc = tc.nc
    vec = nc.vector
    sca = nc.scalar
    mm = nc.tensor.matmul

    const = ctx.enter_context(tc.tile_pool(name="const", bufs=1))
    wpool = ctx.enter_context(tc.tile_pool(name="w", bufs=2))
    xpool = ctx.enter_context(tc.tile_pool(name="x", bufs=2))
    qkpool = ctx.enter_context(tc.tile_pool(name="qk", bufs=2))
    vpool = ctx.enter_context(tc.tile_pool(name="v", bufs=2))
    rpool = ctx.enter_context(tc.tile_pool(name="rope", bufs=2))
    epool = ctx.enter_context(tc.tile_pool(name="e", bufs=4))
    opool = ctx.enter_context(tc.tile_pool(name="o", bufs=1))
    poutp = ctx.enter_context(tc.tile_pool(name="pout", bufs=4))
    ps = ctx.enter_context(tc.tile_pool(name="ps", bufs=1, space="PSUM"))

    cosT = const.tile([P, N], F32)
    sinT = const.tile([P, N], F32)
    tri = const.tile([P, P], BF16)
    ones = const.tile([P, P], BF16)
    warmz = const.tile([P, P], BF16)
    # consts go on the gpsimd DMA queue so they don't delay the first
    # weight/x loads on the sync queue
    nc.gpsimd.dma_start(cosT, t["cosT"])
    nc.gpsimd.dma_start(sinT, t["sinT"])
    nc.gpsimd.dma_start(tri, t["tri"])
    vec.memset(ones, 1.0)
    vec.memset(warmz, 0.0)

    # warm up the PE clock (HAM un-throttles after ~3.4us of activity)
    # while the first weight/x DMAs are in flight
    warm_ps = ps.tile([P, P], F32, tag="PO0", name="warm_ps")
    for _ in range(56):
        mm(warm_ps, warmz, warmz, skip_group_check=True)

    # wp4 loaded later (during sweep 0) to keep the startup DMA short
    wp4 = const.tile([P, HPG, N], BF16, name="wp4")

    outT = [
        opool.tile([P, N], BF16, tag=f"outT{h}", name=f"outT{h}")
        for h in range(HPG)
    ]


    def rope_evac(psrc, which):
        """One fast ACT copy frees the q/k PSUM banks."""
        raw = rpool.tile([P, 2, PANEL], F32, tag=f"raw{which}", name="raw")
        sca.copy(raw, psrc)
        return raw

    def rope_finish(raw, dst, sl):
        """dst[:, :, sl] = raw*cos + swap64(raw)*sin'  (sin' pre-negated on
        partitions 0-63)."""
        rw = rpool.tile([P, 2, PANEL], F32, tag="rw", name="rw")
        sca.copy(rw[0:64], raw[64:128])
        sca.copy(rw[64:128], raw[0:64])
        vec.tensor_mul(rw, rw, _bc2(sinT[:, sl]))
        vec.tensor_mul(raw, raw, _bc2(cosT[:, sl]))
        vec.tensor_add(dst[:, :, sl], raw, rw)

    def load_w(sweep):
        """Per-sweep 256-col slices of wq/wk/wv -> [128, KB, 256] bf16."""
        w_sb = {}
        for wname in ("wq", "wk", "wv"):
            w3 = t[wname][sweep].rearrange("p (kb f) -> p kb f", kb=KB)
            # split into halves so the first matmuls can start early
            w_t = wpool.tile([P, KB, 256], BF16, tag=wname, name=wname)
            for hf in range(2):
                nc.sync.dma_start(
                    w_t[:, 8 * hf : 8 * hf + 8, :],
                    w3[:, 8 * hf : 8 * hf + 8, :],
                )
            w_sb[wname] = w_t
        return w_sb

    for sweep in range(2):
        w_sb = load_w(sweep) if sweep == 0 else w_next  # noqa: F821
        # per-sweep state
        qq = qkpool.tile([P, 2, N], BF16, tag="qq", name="qq")
        kk = qkpool.tile([P, 2, N], BF16, tag="kk", name="kk")
        v_sb = vpool.tile([P, NJB, 2, P], BF16, tag="v", name="v_sb")

        # ---- phase A: QKV + RoPE for this sweep's 2 heads ----
        for p in range(NP):
            sl = slice(PANEL * p, PANEL * (p + 1))
            pq01 = ps.tile([P, 2, PANEL], F32, tag="SC0", name="pq01")
            pk01 = ps.tile([P, 2, PANEL], F32, tag="SC1", name="pk01")
            pv = [
                ps.tile([P, 256], F32, tag=f"PO{tb}", name=f"pv{tb}")
                for tb in range(4)
            ]
            xt = [None, None]
            for hb in range(2):
                xt[hb] = xpool.tile([P, KB // 2, PANEL], BF16, tag="x", name="xt")
                xsrc = t["xTile"][2 * p + hb].rearrange(
                    "q (kb n) -> q kb n", kb=KB // 2
                )
                (nc.gpsimd if hb == 0 else nc.sync).dma_start(xt[hb], xsrc)
            # q,k,v grouped per xt half: q/k PSUM banks drain while the later
            # groups stream, so the next panel never stalls on RoPE; each
            # half only needs its own xt DMA.
            for hb in range(2):
                for wn, pdst in (("wq", pq01), ("wk", pk01)):
                    for kbl in range(KB // 2):
                        kb = 8 * hb + kbl
                        st, sp = kb == 0, kb == KB - 1
                        x_k = xt[hb][:, kbl]
                        mm(pdst[:, 0], w_sb[wn][:, kb, 0:128], x_k, start=st, stop=sp)
                        mm(pdst[:, 1], w_sb[wn][:, kb, 128:256], x_k, start=st, stop=sp)
                    if hb == 1 and wn == "wq":
                        raw_q = rope_evac(pq01, "q")
                for kbl in range(KB // 2):
                    kb = 8 * hb + kbl
                    st, sp = kb == 0, kb == KB - 1
                    x_k = xt[hb][:, kbl]
                    for tb in range(4):
                        mm(
                            pv[tb],
                            x_k[:, 128 * tb : 128 * (tb + 1)],
                            w_sb["wv"][:, kb],
                            start=st,
                            stop=sp,
                        )
                if hb == 1:
                    raw_k = rope_evac(pk01, "k")
            for tb in range(4):
                sca.copy(
                    v_sb[:, 4 * p + tb],
                    pv[tb].rearrange("p (h f) -> p h f", h=2),
                )
            rope_finish(raw_q, qq, sl)
            rope_finish(raw_k, kk, sl)

        # prefetch next sweep's weights / wp4 during attention
        if sweep == 0:
            w_next = load_w(1)
            nc.sync.dma_start(wp4, t["wp4"])

        # ---- phase B: causal SDPA (+ proj on sweep 1) ----
        def emit_proj(pp, tc):
            # proj of panel pp, token chunk tc (all 4 heads' outT final)
            tsl = slice(PANEL * pp + 128 * tc, PANEL * pp + 128 * (tc + 1))
            for occ in range(4):
                pj = ps.tile([P, PANEL], F32, tag=f"SC{occ % 2}", name="pj")
                osl = slice(PANEL * occ, PANEL * (occ + 1))
                for h in range(HPG):
                    mm(
                        pj,
                        outT[h][:, tsl],
                        wp4[:, h, osl],
                        start=(h == 0),
                        stop=(h == HPG - 1),
                    )
                o_t = poutp.tile([P, PANEL], BF16, tag="pout", name="o_t")
                if occ % 2 == 0:
                    sca.copy(o_t, pj)
                else:
                    vec.tensor_copy(o_t, pj)
                (nc.sync if occ % 2 else nc.scalar).dma_start(
                    t["projB"][tsl, osl], o_t
                )

        for p in range(NP):
            njb = 4 * p + 4
            sl = slice(PANEL * p, PANEL * (p + 1))
            po = {
                hh: ps.tile([P, PANEL], F32, tag=f"PO{hh}", name="po")
                for hh in range(2)
            }
            prs = {
                hh: ps.tile([P, PANEL], F32, tag=f"PO{2 + hh}", name="prs")
                for hh in range(2)
            }
            e_tiles = []

            def emit_av(jj):
                # attn@v (stationary = v chunk, long moving stream) plus the
                # rowsum matmul against all-ones; both accumulate over jj.
                e1, n0 = e_tiles[jj]
                st, sp = jj == 0, jj == njb - 1
                for hh in range(2):
                    mm(
                        po[hh][:, n0:],
                        v_sb[:, jj, hh],
                        e1[:, hh, n0:],
                        start=st,
                        stop=sp,
                    )
                    mm(
                        prs[hh][:, n0:], ones, e1[:, hh, n0:], start=st, stop=sp
                    )

            for jj in range(njb):
                td = jj - 4 * p  # diagonal sub-block index if >= 0
                n0 = 128 * td if td > 0 else 0
                sc = ps.tile([P, 2, PANEL], F32, tag=f"SC{jj % 2}", name="sc")
                for hh in range(2):
                    mm(
                        sc[:, hh, n0:],
                        kk[:, hh, 128 * jj : 128 * (jj + 1)],
                        qq[:, hh, PANEL * p + n0 : PANEL * (p + 1)],
                    )
                e1 = epool.tile([P, 2, PANEL], BF16, tag="e1", name="e1")
                sca.activation(e1[:, :, n0:], sc[:, :, n0:], EXP, scale=SCALE)
                if td >= 0:
                    dsl = slice(128 * td, 128 * (td + 1))
                    vec.tensor_mul(e1[:, :, dsl], e1[:, :, dsl], _bc2(tri))
                e_tiles.append((e1, n0))
                if jj >= 2:
                    emit_av(jj - 2)
                    # previous panel's proj interleaves here (sweep 1): by
                    # now its normalize has long finished, so the proj
                    # matmuls never stall the PE at the panel boundary
                    if sweep == 1 and p > 0 and 2 <= jj <= 5:
                        emit_proj(p - 1, jj - 2)
            emit_av(njb - 2)
            emit_av(njb - 1)

            # outT = po / rowsum, straight into [hd, tok] layout
            for hh in range(2):
                rcp = rpool.tile([P, PANEL], F32, tag="rcp", name="rcp")
                vec.reciprocal_approx_fast(rcp, prs[hh])
                vec.tensor_mul(outT[2 * sweep + hh][:, sl], po[hh], rcp)

        if sweep == 1:
            for tc in range(4):
                emit_proj(NP - 1, tc)

    if DEBUG:
        for h in range(HPG):
            nc.sync.dma_start(t[f"dbg_o{h}"], outT[h])


def build_nc():
    key = (DEBUG,)
    if key in _NC_CACHE:
        return _NC_CACHE[key]
    nc = bacc.Bacc("TRN2", target_bir_lowering=False, debug=False)
    t = {}
    t["xTile"] = nc.dram_tensor(
        "xTile", [2 * NP, P, (KB // 2) * PANEL], BF16, kind="ExternalInput"
    ).ap()
    t["wq"] = nc.dram_tensor("wq", [2, P, KB * 256], BF16, kind="ExternalInput").ap()
    t["wk"] = nc.dram_tensor("wk", [2, P, KB * 256], BF16, kind="ExternalInput").ap()
    t["wv"] = nc.dram_tensor("wv", [2, P, KB * 256], BF16, kind="ExternalInput").ap()
    t["wp4"] = nc.dram_tensor("wp4", [P, HPG, N], BF16, kind="ExternalInput").ap()
    t["cosT"] = nc.dram_tensor("cosT", [P, N], F32, kind="ExternalInput").ap()
    t["sinT"] = nc.dram_tensor("sinT", [P, N], F32, kind="ExternalInput").ap()
    t["tri"] = nc.dram_tensor("tri", [P, P], BF16, kind="ExternalInput").ap()
    t["projB"] = nc.dram_tensor("projB", [N, N], BF16, kind="ExternalOutput").ap()
    if DEBUG:
        for h in range(HPG):
            t[f"dbg_o{h}"] = nc.dram_tensor(
                f"dbg_o{h}", [P, N], BF16, kind="ExternalOutput"
            ).ap()
    with tile.TileContext(nc) as tc, ExitStack() as ctx:
        _emit(ctx, tc, t)
    nc.compile()
    _NC_CACHE[key] = nc
    return nc


def make_in_maps(x, position_ids, Wqkv, Wproj):
    x = np.asarray(x, dtype=np.float32)
    pos = np.asarray(position_ids, dtype=np.float64)
    Wqkv = np.asarray(Wqkv, dtype=np.float32)
    Wproj = np.asarray(Wproj, dtype=np.float32)
    bf = ml_dtypes.bfloat16

    inv_freq = 1.0 / (
        ROPE_BASE ** (np.arange(0, HD, 2, dtype=np.float32) / HD)
    )  # [64]
    tri = (np.arange(P)[None, :] >= np.arange(P)[:, None]).astype(bf)

    def xtile(xb):
        # [p, hb, q, kbl, t] tiling of x^T so each [128, 8*512] DMA is
        # contiguous per partition
        xT = np.ascontiguousarray(xb.T)  # [C, N]
        v = xT.reshape(2, 8, P, NP, PANEL).transpose(3, 0, 2, 1, 4)
        return np.ascontiguousarray(v.reshape(2 * NP, P, 8 * PANEL)).astype(bf)

    def wtile(w):
        # [sweep, p, kb, f] tiling of a [C, 512] weight slice
        v = w.reshape(KB, P, 2, 256).transpose(2, 1, 0, 3)
        return np.ascontiguousarray(v.reshape(2, P, KB * 256)).astype(bf)

    in_maps = []
    for c in range(8):
        b, g = divmod(c, G)
        freqs = pos[b].astype(np.float32)[:, None] * inv_freq[None, :]  # [N, 64]
        emb = np.concatenate([freqs, freqs], axis=-1)  # [N, 128]
        cosT = np.ascontiguousarray(np.cos(emb).T)  # [128, N]
        sinT = np.ascontiguousarray(np.sin(emb).T)
        sinT[:64] = -sinT[:64]
        wp4 = np.ascontiguousarray(
            Wproj[512 * g : 512 * (g + 1), :]
            .reshape(HPG, P, N)
            .transpose(1, 0, 2)
        ).astype(bf)
        in_maps.append(
            {
                "xTile": xtile(x[b]),
                "wq": wtile(Wqkv[:, 512 * g : 512 * (g + 1)]),
                "wk": wtile(Wqkv[:, 2048 + 512 * g : 2048 + 512 * (g + 1)]),
                "wv": wtile(Wqkv[:, 4096 + 512 * g : 4096 + 512 * (g + 1)]),
                "wp4": wp4,
                "cosT": cosT,
                "sinT": sinT,
                "tri": tri,
            }
        )
    return in_maps


def kernel(x, position_ids, Wqkv, Wproj, _trace=False, _tmpdir=None):
    nc = build_nc()
    in_maps = make_in_maps(x, position_ids, Wqkv, Wproj)
    res = bass_utils.run_bass_kernel_spmd(
        nc, in_maps, core_ids=list(range(8)), trace=_trace, tmpdir=_tmpdir
    )
    out = np.empty((B, N, C), dtype=np.float32)
    for b in range(B):
        acc = res.results[4 * b]["projB"].astype(np.float32)
        for g in range(1, G):
            acc += res.results[4 * b + g]["projB"].astype(np.float32)
        out[b] = acc
    kernel.last_exec_time_ns = res.exec_time_ns
    kernel.last_results = res
    return out


# revision 30
# speedup vs baseline: 1.2816x; 1.0299x over previous
"""Causal attention block (QKV proj + RoPE + causal SDPA + out proj) on 8
Trainium2 NeuronCores.

Sharding: core c = 4*b + g handles batch b (of 2) and head group g (of 4,
4 heads each).  Each core computes q/k/v for its 4 heads from x[b] and the
matching Wqkv column slices, runs causal SDPA, and contracts its 512
output-feature rows of Wproj, producing a partial projB [2048(tok),
2048(oc)].  The host sums the 4 partials per batch.

v2 design notes (vs the fp32r baseline):
  * All matmul operands are bf16 (PSUM accumulation stays fp32).  bf16
    stationaries enable Fast Weight Load (2 cols/cycle) -- fp32r LDWEIGHTS
    measured ~190ns/tile and made QKV LDW-port-bound (~224ns/MM observed vs
    160ns stream-ideal).  End-to-end bf16 error measured 3.9e-3 <= 2e-2.
  * Softmax row sums are fused into the attn@v matmuls: e-tiles are the
    STATIONARY operand ([128 keys, 128 queries] chunks) and the moving
    operand is v with an appended all-ones column [128 keys, 129].  The
    PSUM result is [queries, hd | rowsum], so the denominator lands as a
    per-partition scalar: reciprocal on [128,1] + tensor_scalar broadcast.
    This removes the separate all-ones rowsum matmul (1/3 of attention
    matmul rows in the baseline).
  * The [q, hd] attention output is transposed back to [hd, q] for the
    projection with SBUF->SBUF dma_start_transpose (xbar), costing no
    engine time.
  * Projection emits projB[tok, oc] (stationary = outT chunk, moving = Wproj
    rows), so neither device nor host transposes the output; output is bf16
    (halves the output DMA).
  * exp runs once per (panel, jb) over both heads' scores ([128, 2, 512-n0]
    strided PSUM read) halving ACT instruction overheads.
  * RoPE reads q/k PSUM directly (swap-halves via partition-offset ACT
    copies + in-place DVE muls); q/k/v PSUM banks free early so the next
    panel's matmuls are never blocked on the RoPE chain.
"""

import sys

if "/opt/trn_rl_repo" not in sys.path:
    sys.path.insert(0, "/opt/trn_rl_repo")

from contextlib import ExitStack

import ml_dtypes
import numpy as np

import concourse.bass as bass  # noqa: F401
import concourse.tile as tile
from concourse import bacc, bass_utils, mybir

F32 = mybir.dt.float32
BF16 = mybir.dt.bfloat16
EXP = mybir.ActivationFunctionType.Exp

B, N, C = 2, 2048, 2048
H = 16  # total heads
HD = C // H  # 128
G = 4  # head groups (cores per batch)
HPG = H // G  # 4 heads per group
P = 128
PANEL = 512
NP = N // PANEL  # 4 token panels
KB = C // P  # 16 contraction blocks
NJB = N // P  # 16 key blocks
SCALE = float(HD) ** -0.5
ROPE_BASE = 10000.0

_NC_CACHE = {}
DEBUG = False


def _bc2(ap, n=2):
    """Broadcast a [128, F] AP across an inserted middle dim -> [128, n, F]."""
    p, f = ap.shape
    return ap.rearrange("p (o n) -> p o n", o=1).broadcast_to([p, n, f])


def _emit(ctx, tc, t):
    nc = tc.nc
    vec = nc.vector
    sca = nc.scalar
    mm = nc.tensor.matmul

    const = ctx.enter_context(tc.tile_pool(name="const", bufs=1))
    wpool = ctx.enter_context(tc.tile_pool(name="w", bufs=2))
    xpool = ctx.enter_context(tc.tile_pool(name="x", bufs=4))
    qkpool = ctx.enter_context(tc.tile_pool(name="qk", bufs=2))
    vpool = ctx.enter_context(tc.tile_pool(name="v", bufs=2))
    rpool = ctx.enter_context(tc.tile_pool(name="rope", bufs=2))
    epool = ctx.enter_context(tc.tile_pool(name="e", bufs=4))
    opool = ctx.enter_context(tc.tile_pool(name="o", bufs=1))
    poutp = ctx.enter_context(tc.tile_pool(name="pout", bufs=4))
    ps = ctx.enter_context(tc.tile_pool(name="ps", bufs=1, space="PSUM"))

    cosT = const.tile([P, N], BF16)
    sinT = const.tile([P, N], BF16)
    tri = const.tile([P, P], BF16)
    ones = const.tile([P, P], BF16)
    warmz = const.tile([P, P], BF16)
    vec.memset(ones, 1.0)
    vec.memset(warmz, 0.0)

    def load_consts():
        # gpsimd DMA queue, emitted after the first x tile so the very first
        # matmuls aren't stuck behind 2MB of cos/sin
        nc.gpsimd.dma_start(cosT, t["cosT"])
        nc.gpsimd.dma_start(sinT, t["sinT"])
        nc.gpsimd.dma_start(tri, t["tri"])

    # warm up the PE clock (HAM un-throttles after ~3.4us of activity)
    # while the first weight/x DMAs are in flight
    warm_ps = ps.tile([P, P], F32, tag="PO0", name="warm_ps")
    for _ in range(56):
        mm(warm_ps, warmz, warmz, skip_group_check=True)

    # wp4 loaded later (during sweep 0) to keep the startup DMA short
    wp4 = const.tile([P, HPG, N], BF16, name="wp4")

    outT = [
        opool.tile([P, N], BF16, tag=f"outT{h}", name=f"outT{h}")
        for h in range(HPG)
    ]


    def rope_evac(psrc, which):
        """One fast ACT copy frees the q/k PSUM banks."""
        raw = rpool.tile([P, 2, PANEL], BF16, tag=f"raw{which}", name="raw")
        sca.copy(raw, psrc)
        return raw

    def rope_finish(raw, dst, sl):
        """dst[:, :, sl] = raw*cos + swap64(raw)*sin'  (sin' pre-negated on
        partitions 0-63)."""
        rw = rpool.tile([P, 2, PANEL], BF16, tag="rw", name="rw")
        sca.copy(rw[0:64], raw[64:128])
        sca.copy(rw[64:128], raw[0:64])
        vec.tensor_mul(rw, rw, _bc2(sinT[:, sl]))
        vec.tensor_mul(raw, raw, _bc2(cosT[:, sl]))
        vec.tensor_add(dst[:, :, sl], raw, rw)

    def load_w(sweep):
        """Per-sweep 256-col slices of wq/wk/wv -> [128, KB, 256] bf16."""
        w_sb = {}
        for wname in ("wq", "wk", "wv"):
            w3 = t[wname][sweep].rearrange("p (kb f) -> p kb f", kb=KB)
            # split into halves so the first matmuls can start early
            w_t = wpool.tile([P, KB, 256], BF16, tag=wname, name=wname)
            for hf in range(2):
                nc.sync.dma_start(
                    w_t[:, 8 * hf : 8 * hf + 8, :],
                    w3[:, 8 * hf : 8 * hf + 8, :],
                )
            w_sb[wname] = w_t
        return w_sb

    for sweep in range(2):
        w_sb = load_w(sweep) if sweep == 0 else w_next  # noqa: F821
        # per-sweep state
        qq = qkpool.tile([P, 2, N], BF16, tag="qq", name="qq")
        kk = qkpool.tile([P, 2, N], BF16, tag="kk", name="kk")
        v_sb = vpool.tile([P, NJB, 2, P], BF16, tag="v", name="v_sb")
        deferred_rope = None

        # ---- phase A: QKV + RoPE for this sweep's 2 heads ----
        for p in range(NP):
            sl = slice(PANEL * p, PANEL * (p + 1))
            pq01 = ps.tile([P, 2, PANEL], F32, tag="SC0", name="pq01")
            pk01 = ps.tile([P, 2, PANEL], F32, tag="SC1", name="pk01")
            pv = [
                ps.tile([P, 256], F32, tag=f"PO{tb}", name=f"pv{tb}")
                for tb in range(4)
            ]
            xt = [None, None]
            for hb in range(2):
                xt[hb] = xpool.tile([P, KB // 2, PANEL], BF16, tag="x", name="xt")
                xsrc = t["xTile"][2 * p + hb].rearrange(
                    "q (kb n) -> q kb n", kb=KB // 2
                )
                (nc.gpsimd if hb == 0 else nc.sync).dma_start(xt[hb], xsrc)
                if sweep == 0 and p == 0 and hb == 0:
                    load_consts()
            # q,k,v grouped per xt half: q/k PSUM banks drain while the later
            # groups stream, so the next panel never stalls on RoPE; each
            # half only needs its own xt DMA.
            for hb in range(2):
                for wn, pdst in (("wq", pq01), ("wk", pk01)):
                    for kbl in range(KB // 2):
                        kb = 8 * hb + kbl
                        st, sp = kb == 0, kb == KB - 1
                        x_k = xt[hb][:, kbl]
                        mm(pdst[:, 0], w_sb[wn][:, kb, 0:128], x_k, start=st, stop=sp)
                        mm(pdst[:, 1], w_sb[wn][:, kb, 128:256], x_k, start=st, stop=sp)
                    if hb == 1 and wn == "wq":
                        raw_q = rope_evac(pq01, "q")
                for kbl in range(KB // 2):
                    kb = 8 * hb + kbl
                    st, sp = kb == 0, kb == KB - 1
                    x_k = xt[hb][:, kbl]
                    for tb in range(4):
                        mm(
                            pv[tb],
                            x_k[:, 128 * tb : 128 * (tb + 1)],
                            w_sb["wv"][:, kb],
                            start=st,
                            stop=sp,
                        )
                if hb == 1:
                    raw_k = rope_evac(pk01, "k")
            for tb in range(4):
                vec.tensor_copy(
                    v_sb[:, 4 * p + tb],
                    pv[tb].rearrange("p (h f) -> p h f", h=2),
                )
            if p < NP - 1:
                rope_finish(raw_q, qq, sl)
                rope_finish(raw_k, kk, sl)
            else:
                deferred_rope = (raw_q, raw_k, sl)

        # prefetch next sweep's weights / wp4 during attention
        if sweep == 0:
            w_next = load_w(1)
            nc.sync.dma_start(wp4, t["wp4"])

        # ---- phase B: causal SDPA (+ proj on sweep 1) ----
        def emit_proj(pp, tc):
            # proj of panel pp, token chunk tc (all 4 heads' outT final)
            tsl = slice(PANEL * pp + 128 * tc, PANEL * pp + 128 * (tc + 1))
            for occ in range(4):
                pj = ps.tile([P, PANEL], F32, tag=f"SC{occ % 2}", name="pj")
                osl = slice(PANEL * occ, PANEL * (occ + 1))
                for h in range(HPG):
                    mm(
                        pj,
                        outT[h][:, tsl],
                        wp4[:, h, osl],
                        start=(h == 0),
                        stop=(h == HPG - 1),
                    )
                o_t = poutp.tile([P, PANEL], BF16, tag="pout", name="o_t")
                if occ % 2 == 0:
                    sca.copy(o_t, pj)
                else:
                    vec.tensor_copy(o_t, pj)
                (nc.sync if occ % 2 else nc.scalar).dma_start(
                    t["projB"][tsl, osl], o_t
                )

        for p in range(NP):
            njb = 4 * p + 4
            sl = slice(PANEL * p, PANEL * (p + 1))
            po = {
                hh: ps.tile([P, PANEL], F32, tag=f"PO{hh}", name="po")
                for hh in range(2)
            }
            prs = {
                hh: ps.tile([P, PANEL], F32, tag=f"PO{2 + hh}", name="prs")
                for hh in range(2)
            }
            e_tiles = []

            def emit_av(jj):
                # attn@v (stationary = v chunk, long moving stream) plus the
                # rowsum matmul against all-ones; both accumulate over jj.
                e1, n0 = e_tiles[jj]
                st, sp = jj == 0, jj == njb - 1
                for hh in range(2):
                    mm(
                        po[hh][:, n0:],
                        v_sb[:, jj, hh],
                        e1[:, hh, n0:],
                        start=st,
                        stop=sp,
                    )
                    mm(
                        prs[hh][:, n0:], ones, e1[:, hh, n0:], start=st, stop=sp
                    )

            for jj in range(njb):
                td = jj - 4 * p  # diagonal sub-block index if >= 0
                n0 = 128 * td if td > 0 else 0
                sc = ps.tile([P, 2, PANEL], F32, tag=f"SC{jj % 2}", name="sc")
                for hh in range(2):
                    mm(
                        sc[:, hh, n0:],
                        kk[:, hh, 128 * jj : 128 * (jj + 1)],
                        qq[:, hh, PANEL * p + n0 : PANEL * (p + 1)],
                    )
                e1 = epool.tile([P, 2, PANEL], BF16, tag="e1", name="e1")
                sca.activation(e1[:, :, n0:], sc[:, :, n0:], EXP, scale=SCALE)
                if td >= 0:
                    dsl = slice(128 * td, 128 * (td + 1))
                    vec.tensor_mul(e1[:, :, dsl], e1[:, :, dsl], _bc2(tri))
                e_tiles.append((e1, n0))
                if p == 0 and jj == 0 and deferred_rope is not None:
                    dq, dk, dsl2 = deferred_rope
                    rope_finish(dq, qq, dsl2)
                    rope_finish(dk, kk, dsl2)
                    deferred_rope = None
                if jj >= 2:
                    emit_av(jj - 2)
                    # previous panel's proj interleaves here (sweep 1): by
                    # now its normalize has long finished, so the proj
                    # matmuls never stall the PE at the panel boundary
                    if sweep == 1 and p > 0 and 2 <= jj <= 5:
                        emit_proj(p - 1, jj - 2)
            emit_av(njb - 2)
            emit_av(njb - 1)

            # outT = po / rowsum, straight into [hd, tok] layout
            for hh in range(2):
                rcp = rpool.tile([P, PANEL], F32, tag="rcp", name="rcp")
                vec.reciprocal_approx_fast(rcp, prs[hh])
                vec.tensor_mul(outT[2 * sweep + hh][:, sl], po[hh], rcp)

        if sweep == 1:
            for tc in range(4):
                emit_proj(NP - 1, tc)

    if DEBUG:
        for h in range(HPG):
            nc.sync.dma_start(t[f"dbg_o{h}"], outT[h])


def build_nc():
    key = (DEBUG,)
    if key in _NC_CACHE:
        return _NC_CACHE[key]
    nc = bacc.Bacc("TRN2", target_bir_lowering=False, debug=False)
    t = {}
    t["xTile"] = nc.dram_tensor(
        "xTile", [2 * NP, P, (KB // 2) * PANEL], BF16, kind="ExternalInput"
    ).ap()
    t["wq"] = nc.dram_tensor("wq", [2, P, KB * 256], BF16, kind="ExternalInput").ap()
    t["wk"] = nc.dram_tensor("wk", [2, P, KB * 256], BF16, kind="ExternalInput").ap()
    t["wv"] = nc.dram_tensor("wv", [2, P, KB * 256], BF16, kind="ExternalInput").ap()
    t["wp4"] = nc.dram_tensor("wp4", [P, HPG, N], BF16, kind="ExternalInput").ap()
    t["cosT"] = nc.dram_tensor("cosT", [P, N], BF16, kind="ExternalInput").ap()
    t["sinT"] = nc.dram_tensor("sinT", [P, N], BF16, kind="ExternalInput").ap()
    t["tri"] = nc.dram_tensor("tri", [P, P], BF16, kind="ExternalInput").ap()
    t["projB"] = nc.dram_tensor("projB", [N, N], BF16, kind="ExternalOutput").ap()
    if DEBUG:
        for h in range(HPG):
            t[f"dbg_o{h}"] = nc.dram_tensor(
                f"dbg_o{h}", [P, N], BF16, kind="ExternalOutput"
            ).ap()
    with tile.TileContext(nc) as tc, ExitStack() as ctx:
        _emit(ctx, tc, t)
    nc.compile()
    _NC_CACHE[key] = nc
    return nc


def make_in_maps(x, position_ids, Wqkv, Wproj):
    x = np.asarray(x, dtype=np.float32)
    pos = np.asarray(position_ids, dtype=np.float64)
    Wqkv = np.asarray(Wqkv, dtype=np.float32)
    Wproj = np.asarray(Wproj, dtype=np.float32)
    bf = ml_dtypes.bfloat16

    inv_freq = 1.0 / (
        ROPE_BASE ** (np.arange(0, HD, 2, dtype=np.float32) / HD)
    )  # [64]
    tri = (np.arange(P)[None, :] >= np.arange(P)[:, None]).astype(bf)

    def xtile(xb):
        # [p, hb, q, kbl, t] tiling of x^T so each [128, 8*512] DMA is
        # contiguous per partition
        xT = np.ascontiguousarray(xb.T)  # [C, N]
        v = xT.reshape(2, 8, P, NP, PANEL).transpose(3, 0, 2, 1, 4)
        return np.ascontiguousarray(v.reshape(2 * NP, P, 8 * PANEL)).astype(bf)

    def wtile(w):
        # [sweep, p, kb, f] tiling of a [C, 512] weight slice
        v = w.reshape(KB, P, 2, 256).transpose(2, 1, 0, 3)
        return np.ascontiguousarray(v.reshape(2, P, KB * 256)).astype(bf)

    in_maps = []
    for c in range(8):
        b, g = divmod(c, G)
        freqs = pos[b].astype(np.float32)[:, None] * inv_freq[None, :]  # [N, 64]
        emb = np.concatenate([freqs, freqs], axis=-1)  # [N, 128]
        cosT = np.ascontiguousarray(np.cos(emb).T).astype(bf)  # [128, N]
        sinT = np.ascontiguousarray(np.sin(emb).T)
        sinT[:64] = -sinT[:64]
        sinT = sinT.astype(bf)
        wp4 = np.ascontiguousarray(
            Wproj[512 * g : 512 * (g + 1), :]
            .reshape(HPG, P, N)
            .transpose(1, 0, 2)
        ).astype(bf)
        in_maps.append(
            {
                "xTile": xtile(x[b]),
                "wq": wtile(Wqkv[:, 512 * g : 512 * (g + 1)]),
                "wk": wtile(Wqkv[:, 2048 + 512 * g : 2048 + 512 * (g + 1)]),
                "wv": wtile(Wqkv[:, 4096 + 512 * g : 4096 + 512 * (g + 1)]),
                "wp4": wp4,
                "cosT": cosT,
                "sinT": sinT,
                "tri": tri,
            }
        )
    return in_maps


def kernel(x, position_ids, Wqkv, Wproj, _trace=False, _tmpdir=None):
    nc = build_nc()
    in_maps = make_in_maps(x, position_ids, Wqkv, Wproj)
    res = bass_utils.run_bass_kernel_spmd(
        nc, in_maps, core_ids=list(range(8)), trace=_trace, tmpdir=_tmpdir
    )
    out = np.empty((B, N, C), dtype=np.float32)
    for b in range(B):
        acc = res.results[4 * b]["projB"].astype(np.float32)
        for g in range(1, G):
            acc += res.results[4 * b + g]["projB"].astype(np.float32)
        out[b] = acc
    kernel.last_exec_time_ns = res.exec_time_ns
    kernel.last_results = res
    return out


# revision 31
# speedup vs baseline: 1.2817x; 1.0000x over previous
"""Causal attention block (QKV proj + RoPE + causal SDPA + out proj) on 8
Trainium2 NeuronCores.

Sharding: core c = 4*b + g handles batch b (of 2) and head group g (of 4,
4 heads each).  Each core computes q/k/v for its 4 heads from x[b] and the
matching Wqkv column slices, runs causal SDPA, and contracts its 512
output-feature rows of Wproj, producing a partial projB [2048(tok),
2048(oc)].  The host sums the 4 partials per batch (no transposes
anywhere: attention output is produced as [hd, tok] and the projection
emits [tok, oc] directly).

Design notes (vs the fp32r v1 baseline, 549us -> ~356us traced):
  * All matmul operands are bf16 (PSUM accumulation stays fp32).  fp32r
    LDWEIGHTS measured ~190ns/tile which made QKV LDWEIGHTS-port-bound;
    bf16 stationaries load in ~98ns.  End-to-end error 4.1e-3 << 2e-2.
  * Per 512-token panel, QKV matmuls run grouped q(hb0) k(hb0) v(hb0)
    q(hb1) k(hb1) v(hb1): each group's PSUM banks drain (one fast ACT
    copy) while later groups stream, so the next panel never stalls, and
    each half-panel only needs its own 1MB x DMA (contiguous host tiling,
    split across the sync/gpsimd queues).
  * RoPE = ACT evacuation copy + 64-partition-swap copies + 3 DVE ops, all
    in bf16 (2x DVE mode); cos/sin tables bf16 with sin pre-negated on
    partitions 0-63.  The last panel's RoPE-finish is deferred past the
    first exp so attention starts immediately at the phase boundary.
  * Attention: scores for both heads land in one 2-bank PSUM tile, one
    merged exp per (panel, key-block) with the causal diagonal masked by a
    tri multiply; attn@v (stationary v) and the all-ones rowsum matmul
    accumulate per key-block; outT = po * reciprocal_approx_fast(rowsum).
    Softmax skips max-subtraction (|scores| small, exp safe in fp32).
  * Projection: stationary = outT chunk (reused across 4 oc-chunks),
    moving = Wproj rows -> projB[tok, oc], bf16 output (half the DMA).
    Panel p's projection is interleaved into panel p+1's attention loop so
    it never stalls on the normalize chain.
  * ~88 zero matmuls at kernel start warm the PE clock (HAM un-throttles
    after ~3.4us of activity) while the first DMAs are in flight.
  * PSUM discipline: one accumulation group per 2KB bank (a start=True
    matmul clears has_written for the whole bank), and no engine ever
    reads a bank the PE may still write (bank-level collisions are not
    ordered by the address-level dependency tracker and corrupt results
    nondeterministically -- the v2 dma_start_transpose path hit this class
    of race and was replaced).
"""

import sys

if "/opt/trn_rl_repo" not in sys.path:
    sys.path.insert(0, "/opt/trn_rl_repo")

from contextlib import ExitStack

import ml_dtypes
import numpy as np

import concourse.bass as bass  # noqa: F401
import concourse.tile as tile
from concourse import bacc, bass_utils, mybir

F32 = mybir.dt.float32
BF16 = mybir.dt.bfloat16
EXP = mybir.ActivationFunctionType.Exp

B, N, C = 2, 2048, 2048
H = 16  # total heads
HD = C // H  # 128
G = 4  # head groups (cores per batch)
HPG = H // G  # 4 heads per group
P = 128
PANEL = 512
NP = N // PANEL  # 4 token panels
KB = C // P  # 16 contraction blocks
NJB = N // P  # 16 key blocks
SCALE = float(HD) ** -0.5
ROPE_BASE = 10000.0

_NC_CACHE = {}
DEBUG = False


def _bc2(ap, n=2):
    """Broadcast a [128, F] AP across an inserted middle dim -> [128, n, F]."""
    p, f = ap.shape
    return ap.rearrange("p (o n) -> p o n", o=1).broadcast_to([p, n, f])


def _emit(ctx, tc, t):
    nc = tc.nc
    vec = nc.vector
    sca = nc.scalar
    mm = nc.tensor.matmul

    const = ctx.enter_context(tc.tile_pool(name="const", bufs=1))
    wpool = ctx.enter_context(tc.tile_pool(name="w", bufs=2))
    xpool = ctx.enter_context(tc.tile_pool(name="x", bufs=4))
    qkpool = ctx.enter_context(tc.tile_pool(name="qk", bufs=2))
    vpool = ctx.enter_context(tc.tile_pool(name="v", bufs=2))
    rpool = ctx.enter_context(tc.tile_pool(name="rope", bufs=2))
    epool = ctx.enter_context(tc.tile_pool(name="e", bufs=4))
    opool = ctx.enter_context(tc.tile_pool(name="o", bufs=1))
    poutp = ctx.enter_context(tc.tile_pool(name="pout", bufs=4))
    ps = ctx.enter_context(tc.tile_pool(name="ps", bufs=1, space="PSUM"))

    cosT = const.tile([P, N], BF16)
    sinT = const.tile([P, N], BF16)
    tri = const.tile([P, P], BF16)
    ones = const.tile([P, P], BF16)
    warmz = const.tile([P, P], BF16)
    vec.memset(ones, 1.0)
    vec.memset(warmz, 0.0)

    def load_consts():
        # gpsimd DMA queue, emitted after the first x tile so the very first
        # matmuls aren't stuck behind 2MB of cos/sin
        nc.gpsimd.dma_start(cosT, t["cosT"])
        nc.gpsimd.dma_start(sinT, t["sinT"])
        nc.gpsimd.dma_start(tri, t["tri"])

    # warm up the PE clock (HAM un-throttles after ~3.4us of activity)
    # while the first weight/x DMAs are in flight
    warm_ps = ps.tile([P, P], F32, tag="PO0", name="warm_ps")
    for _ in range(88):
        mm(warm_ps, warmz, warmz, skip_group_check=True)

    # wp4 loaded later (during sweep 0) to keep the startup DMA short
    wp4 = const.tile([P, HPG, N], BF16, name="wp4")

    outT = [
        opool.tile([P, N], BF16, tag=f"outT{h}", name=f"outT{h}")
        for h in range(HPG)
    ]


    def rope_evac(psrc, which):
        """One fast ACT copy frees the q/k PSUM banks."""
        raw = rpool.tile([P, 2, PANEL], BF16, tag=f"raw{which}", name="raw")
        sca.copy(raw, psrc)
        return raw

    def rope_finish(raw, dst, sl):
        """dst[:, :, sl] = raw*cos + swap64(raw)*sin'  (sin' pre-negated on
        partitions 0-63)."""
        rw = rpool.tile([P, 2, PANEL], BF16, tag="rw", name="rw")
        sca.copy(rw[0:64], raw[64:128])
        sca.copy(rw[64:128], raw[0:64])
        vec.tensor_mul(rw, rw, _bc2(sinT[:, sl]))
        vec.tensor_mul(raw, raw, _bc2(cosT[:, sl]))
        vec.tensor_add(dst[:, :, sl], raw, rw)

    def load_w(sweep):
        """Per-sweep 256-col slices of wq/wk/wv -> [128, KB, 256] bf16."""
        w_sb = {}
        for wname in ("wq", "wk", "wv"):
            w3 = t[wname][sweep].rearrange("p (kb f) -> p kb f", kb=KB)
            # split into halves so the first matmuls can start early
            w_t = wpool.tile([P, KB, 256], BF16, tag=wname, name=wname)
            for hf in range(2):
                nc.sync.dma_start(
                    w_t[:, 8 * hf : 8 * hf + 8, :],
                    w3[:, 8 * hf : 8 * hf + 8, :],
                )
            w_sb[wname] = w_t
        return w_sb

    for sweep in range(2):
        w_sb = load_w(sweep) if sweep == 0 else w_next  # noqa: F821
        # per-sweep state
        qq = qkpool.tile([P, 2, N], BF16, tag="qq", name="qq")
        kk = qkpool.tile([P, 2, N], BF16, tag="kk", name="kk")
        v_sb = vpool.tile([P, NJB, 2, P], BF16, tag="v", name="v_sb")
        deferred_rope = None

        # ---- phase A: QKV + RoPE for this sweep's 2 heads ----
        for p in range(NP):
            sl = slice(PANEL * p, PANEL * (p + 1))
            pq01 = ps.tile([P, 2, PANEL], F32, tag="SC0", name="pq01")
            pk01 = ps.tile([P, 2, PANEL], F32, tag="SC1", name="pk01")
            pv = [
                ps.tile([P, 256], F32, tag=f"PO{tb}", name=f"pv{tb}")
                for tb in range(4)
            ]
            xt = [None, None]
            for hb in range(2):
                xt[hb] = xpool.tile([P, KB // 2, PANEL], BF16, tag="x", name="xt")
                xsrc = t["xTile"][2 * p + hb].rearrange(
                    "q (kb n) -> q kb n", kb=KB // 2
                )
                (nc.gpsimd if hb == 0 else nc.sync).dma_start(xt[hb], xsrc)
                if sweep == 0 and p == 0 and hb == 0:
                    load_consts()
            # q,k,v grouped per xt half: q/k PSUM banks drain while the later
            # groups stream, so the next panel never stalls on RoPE; each
            # half only needs its own xt DMA.
            for hb in range(2):
                for wn, pdst in (("wq", pq01), ("wk", pk01)):
                    for kbl in range(KB // 2):
                        kb = 8 * hb + kbl
                        st, sp = kb == 0, kb == KB - 1
                        x_k = xt[hb][:, kbl]
                        mm(pdst[:, 0], w_sb[wn][:, kb, 0:128], x_k, start=st, stop=sp)
                        mm(pdst[:, 1], w_sb[wn][:, kb, 128:256], x_k, start=st, stop=sp)
                    if hb == 1 and wn == "wq":
                        raw_q = rope_evac(pq01, "q")
                for kbl in range(KB // 2):
                    kb = 8 * hb + kbl
                    st, sp = kb == 0, kb == KB - 1
                    x_k = xt[hb][:, kbl]
                    for tb in range(4):
                        mm(
                            pv[tb],
                            x_k[:, 128 * tb : 128 * (tb + 1)],
                            w_sb["wv"][:, kb],
                            start=st,
                            stop=sp,
                        )
                if hb == 1:
                    raw_k = rope_evac(pk01, "k")
            for tb in range(4):
                vec.tensor_copy(
                    v_sb[:, 4 * p + tb],
                    pv[tb].rearrange("p (h f) -> p h f", h=2),
                )
            if p < NP - 1:
                rope_finish(raw_q, qq, sl)
                rope_finish(raw_k, kk, sl)
            else:
                deferred_rope = (raw_q, raw_k, sl)

        # prefetch next sweep's weights / wp4 during attention
        if sweep == 0:
            w_next = load_w(1)
            nc.sync.dma_start(wp4, t["wp4"])

        # ---- phase B: causal SDPA (+ proj on sweep 1) ----
        def emit_proj(pp, tc):
            # proj of panel pp, token chunk tc (all 4 heads' outT final)
            tsl = slice(PANEL * pp + 128 * tc, PANEL * pp + 128 * (tc + 1))
            for occ in range(4):
                pj = ps.tile([P, PANEL], F32, tag=f"SC{occ % 2}", name="pj")
                osl = slice(PANEL * occ, PANEL * (occ + 1))
                for h in range(HPG):
                    mm(
                        pj,
                        outT[h][:, tsl],
                        wp4[:, h, osl],
                        start=(h == 0),
                        stop=(h == HPG - 1),
                    )
                o_t = poutp.tile([P, PANEL], BF16, tag="pout", name="o_t")
                if occ % 2 == 0:
                    sca.copy(o_t, pj)
                else:
                    vec.tensor_copy(o_t, pj)
                (nc.sync if occ % 2 else nc.scalar).dma_start(
                    t["projB"][tsl, osl], o_t
                )

        for p in range(NP):
            njb = 4 * p + 4
            sl = slice(PANEL * p, PANEL * (p + 1))
            po = {
                hh: ps.tile([P, PANEL], F32, tag=f"PO{hh}", name="po")
                for hh in range(2)
            }
            prs = {
                hh: ps.tile([P, PANEL], F32, tag=f"PO{2 + hh}", name="prs")
                for hh in range(2)
            }
            e_tiles = []

            def emit_av(jj):
                # attn@v (stationary = v chunk, long moving stream) plus the
                # rowsum matmul against all-ones; both accumulate over jj.
                e1, n0 = e_tiles[jj]
                st, sp = jj == 0, jj == njb - 1
                for hh in range(2):
                    mm(
                        po[hh][:, n0:],
                        v_sb[:, jj, hh],
                        e1[:, hh, n0:],
                        start=st,
                        stop=sp,
                    )
                    mm(
                        prs[hh][:, n0:], ones, e1[:, hh, n0:], start=st, stop=sp
                    )

            for jj in range(njb):
                td = jj - 4 * p  # diagonal sub-block index if >= 0
                n0 = 128 * td if td > 0 else 0
                sc = ps.tile([P, 2, PANEL], F32, tag=f"SC{jj % 2}", name="sc")
                for hh in range(2):
                    mm(
                        sc[:, hh, n0:],
                        kk[:, hh, 128 * jj : 128 * (jj + 1)],
                        qq[:, hh, PANEL * p + n0 : PANEL * (p + 1)],
                    )
                e1 = epool.tile([P, 2, PANEL], BF16, tag="e1", name="e1")
                sca.activation(e1[:, :, n0:], sc[:, :, n0:], EXP, scale=SCALE)
                if td >= 0:
                    dsl = slice(128 * td, 128 * (td + 1))
                    vec.tensor_mul(e1[:, :, dsl], e1[:, :, dsl], _bc2(tri))
                e_tiles.append((e1, n0))
                if p == 0 and jj == 0 and deferred_rope is not None:
                    dq, dk, dsl2 = deferred_rope
                    rope_finish(dq, qq, dsl2)
                    rope_finish(dk, kk, dsl2)
                    deferred_rope = None
                if jj >= 2:
                    emit_av(jj - 2)
                    # previous panel's proj interleaves here (sweep 1): by
                    # now its normalize has long finished, so the proj
                    # matmuls never stall the PE at the panel boundary
                    if sweep == 1 and p > 0 and 2 <= jj <= 5:
                        emit_proj(p - 1, jj - 2)
            emit_av(njb - 2)
            emit_av(njb - 1)

            # outT = po / rowsum, straight into [hd, tok] layout
            for hh in range(2):
                rcp = rpool.tile([P, PANEL], F32, tag="rcp", name="rcp")
                vec.reciprocal_approx_fast(rcp, prs[hh])
                vec.tensor_mul(outT[2 * sweep + hh][:, sl], po[hh], rcp)

        if sweep == 1:
            for tc in range(4):
                emit_proj(NP - 1, tc)

    if DEBUG:
        for h in range(HPG):
            nc.sync.dma_start(t[f"dbg_o{h}"], outT[h])


def build_nc():
    key = (DEBUG,)
    if key in _NC_CACHE:
        return _NC_CACHE[key]
    nc = bacc.Bacc("TRN2", target_bir_lowering=False, debug=False)
    t = {}
    t["xTile"] = nc.dram_tensor(
        "xTile", [2 * NP, P, (KB // 2) * PANEL], BF16, kind="ExternalInput"
    ).ap()
    t["wq"] = nc.dram_tensor("wq", [2, P, KB * 256], BF16, kind="ExternalInput").ap()
    t["wk"] = nc.dram_tensor("wk", [2, P, KB * 256], BF16, kind="ExternalInput").ap()
    t["wv"] = nc.dram_tensor("wv", [2, P, KB * 256], BF16, kind="ExternalInput").ap()
    t["wp4"] = nc.dram_tensor("wp4", [P, HPG, N], BF16, kind="ExternalInput").ap()
    t["cosT"] = nc.dram_tensor("cosT", [P, N], BF16, kind="ExternalInput").ap()
    t["sinT"] = nc.dram_tensor("sinT", [P, N], BF16, kind="ExternalInput").ap()
    t["tri"] = nc.dram_tensor("tri", [P, P], BF16, kind="ExternalInput").ap()
    t["projB"] = nc.dram_tensor("projB", [N, N], BF16, kind="ExternalOutput").ap()
    if DEBUG:
        for h in range(HPG):
            t[f"dbg_o{h}"] = nc.dram_tensor(
                f"dbg_o{h}", [P, N], BF16, kind="ExternalOutput"
            ).ap()
    with tile.TileContext(nc) as tc, ExitStack() as ctx:
        _emit(ctx, tc, t)
    nc.compile()
    _NC_CACHE[key] = nc
    return nc


def make_in_maps(x, position_ids, Wqkv, Wproj):
    x = np.asarray(x, dtype=np.float32)
    pos = np.asarray(position_ids, dtype=np.float64)
    Wqkv = np.asarray(Wqkv, dtype=np.float32)
    Wproj = np.asarray(Wproj, dtype=np.float32)
    bf = ml_dtypes.bfloat16

    inv_freq = 1.0 / (
        ROPE_BASE ** (np.arange(0, HD, 2, dtype=np.float32) / HD)
    )  # [64]
    tri = (np.arange(P)[None, :] >= np.arange(P)[:, None]).astype(bf)

    def xtile(xb):
        # [p, hb, q, kbl, t] tiling of x^T so each [128, 8*512] DMA is
        # contiguous per partition
        xT = np.ascontiguousarray(xb.T)  # [C, N]
        v = xT.reshape(2, 8, P, NP, PANEL).transpose(3, 0, 2, 1, 4)
        return np.ascontiguousarray(v.reshape(2 * NP, P, 8 * PANEL)).astype(bf)

    def wtile(w):
        # [sweep, p, kb, f] tiling of a [C, 512] weight slice
        v = w.reshape(KB, P, 2, 256).transpose(2, 1, 0, 3)
        return np.ascontiguousarray(v.reshape(2, P, KB * 256)).astype(bf)

    in_maps = []
    for c in range(8):
        b, g = divmod(c, G)
        freqs = pos[b].astype(np.float32)[:, None] * inv_freq[None, :]  # [N, 64]
        emb = np.concatenate([freqs, freqs], axis=-1)  # [N, 128]
        cosT = np.ascontiguousarray(np.cos(emb).T).astype(bf)  # [128, N]
        sinT = np.ascontiguousarray(np.sin(emb).T)
        sinT[:64] = -sinT[:64]
        sinT = sinT.astype(bf)
        wp4 = np.ascontiguousarray(
            Wproj[512 * g : 512 * (g + 1), :]
            .reshape(HPG, P, N)
            .transpose(1, 0, 2)
        ).astype(bf)
        in_maps.append(
            {
                "xTile": xtile(x[b]),
                "wq": wtile(Wqkv[:, 512 * g : 512 * (g + 1)]),
                "wk": wtile(Wqkv[:, 2048 + 512 * g : 2048 + 512 * (g + 1)]),
                "wv": wtile(Wqkv[:, 4096 + 512 * g : 4096 + 512 * (g + 1)]),
                "wp4": wp4,
                "cosT": cosT,
                "sinT": sinT,
                "tri": tri,
            }
        )
    return in_maps


def kernel(x, position_ids, Wqkv, Wproj, _trace=False, _tmpdir=None):
    nc = build_nc()
    in_maps = make_in_maps(x, position_ids, Wqkv, Wproj)
    res = bass_utils.run_bass_kernel_spmd(
        nc, in_maps, core_ids=list(range(8)), trace=_trace, tmpdir=_tmpdir
    )
    out = np.empty((B, N, C), dtype=np.float32)
    for b in range(B):
        acc = res.results[4 * b]["projB"].astype(np.float32)
        for g in range(1, G):
            acc += res.results[4 * b + g]["projB"].astype(np.float32)
        out[b] = acc
    kernel.last_exec_time_ns = res.exec_time_ns
    kernel.last_results = res
    return out
